# revision 54
# baseline (speedup 1.0000x reference)
"""Trainium2 Bass kernel for FGAEmbedder (B=32, T=1024, IN=1536, D=768).

Math (identical to the reference up to float reassociation + fp8 noise;
validated vs the jax reference in numpy at rel_err ~1.1e-2 < 2e-2):
    h  = relu(x @ W1^T + b1)           [B,T,IN]   fp8 (x8/w1 fp8, DR)
    u  = h @ W2^T + b2                 [B,T,D]    fp8
    e  = relu(u @ We^T + be)  ; un = e @ Wr^T + br       (score path)
    xe = u @ Wx^T + bx ; ye = u @ Wy^T + by              (score path)
    pw[t] = (xe[t] . ybar) / ||xe[t]||, ybar = sum_s ye[s]/||ye[s]||
    w  = softmax(rw0*un + rw1*pw)
    out = (sum_t w[t] * h[t]) @ W2^T + b2        <- fc2 is linear, so the
          weighted sum is pushed through W2 (two-plane fp8 hi+lo weights).

Perf structure:
  - every big matmul is fp8 DoubleRow, and each stationary weight block is
    loaded ONCE and used for TWO N=512 moving halves (the redundant second
    LDWEIGHTS of an identical stationary hides completely under the matmul,
    ~213ns/MM vs ~300ns when alternating LDW/MM with distinct weights).
  - fc1's ACT evacuation writes h8 (16*h fp8) directly: no fp16 h tensor, no
    DVE quantize pass; the exact-path weighted sum V runs off h8.
  - row reductions (un-reduce, sum-of-squares for the cosine norms) are fp8
    DoubleRow matmuls with 16B-padded dual stationaries.
  - w1 is packed m-major on the host so the fc1 weight stream arrives in
    exactly the order the PE consumes it; x8 is prefetched one batch ahead.

Sharding: data-parallel over batch, 4 batches per core, no collectives.
"""

import numpy as np
import ml_dtypes

import concourse.bass as bass
import concourse.bacc as bacc
import concourse.mybir as mybir
import concourse.tile as tile
from concourse.bass_utils import run_bass_kernel_spmd

# The kernel's ACT functions are Relu/Identity/Copy/Ln/Exp. Ln and Exp only
# coexist in the "natural_log_exp_and_others" table set, but the greedy set
# chooser maps exp->"exp_and_others" and ln->"natural_log", thrashing
# ACT_TABLE_LOADs (1.5us each) all kernel long. Filter Exp/Ln out of every
# other set (names and order — and therefore act_func_set_ids — unchanged)
# so the chooser lands on the combined set: exactly one load at startup.
_orig_gat = bacc.get_activation_tables


def _gat_single_set(arch):
    tables = _orig_gat(arch)
    AFt = mybir.ActivationFunctionType
    out = {}
    for name, fns in tables.items():
        fns = set(fns)
        if name != "natural_log_exp_and_others":
            fns.discard(AFt.Exp)
            fns.discard(AFt.Ln)
        out[name] = fns
    return out


bacc.get_activation_tables = _gat_single_set

B, T, IN, D = 32, 1024, 1536, 768
NCORES = 8
BPC = B // NCORES        # batches per core
NT = 512                 # matmul moving free dim (one PSUM bank)
NTT = T // NT            # moving halves
KI = IN // 128           # 12 feature tiles of the 1536 dim
KIP = KI // 2            # 6 fp8 double-row k-pairs
KD = D // 128            # 6 feature tiles of the 768 dim
KDP = KD // 2            # 3 fp8 double-row k-pairs

SX = 16.0                # fp8 activation scale
SW = 64.0                # fp8 weight scale

F16 = mybir.dt.float16
F8 = mybir.dt.float8e4
F32 = mybir.dt.float32
AF = mybir.ActivationFunctionType
ALU = mybir.AluOpType
AX = mybir.AxisListType
DR = mybir.MatmulPerfMode.DoubleRow


def build_nc(bpc: int = BPC) -> bass.Bass:
    nc = bacc.Bacc()

    # x8 pre-packed partition-major on the host: [b, p, ko, t] so the per-
    # batch DMA is one fully-contiguous 1.5MB transfer (a (ko p) t gather in
    # 1KB chunks only sustains ~140GB/s and starved the first fc1)
    xt = nc.declare_dram_parameter("xt", [bpc, 128, KI, T], F8, isOutput=False)
    # w1 m-major dual pack: [m, 128, kp, 2, 128]
    w1p = nc.declare_dram_parameter("w1p", [KI, 128, KIP, 2, 128], F8,
                                    isOutput=False)
    w2hi = nc.declare_dram_parameter("w2hi", [128, KIP, 2, D], F8,
                                     isOutput=False)
    w2lo = nc.declare_dram_parameter("w2lo", [128, KIP, 2, D], F8,
                                     isOutput=False)
    wet = nc.declare_dram_parameter("wet", [128, KDP, 2, D], F8,
                                    isOutput=False)
    wxt = nc.declare_dram_parameter("wxt", [128, KDP, 2, D], F8,
                                    isOutput=False)
    wyt = nc.declare_dram_parameter("wyt", [128, KDP, 2, D], F8,
                                    isOutput=False)
    wr8d = nc.declare_dram_parameter("wr8", [128, KDP, 2, 16], F8,
                                     isOutput=False)
    ones8d = nc.declare_dram_parameter("ones8", [128, KDP, 2, 16], F8,
                                       isOutput=False)
    b1d = nc.declare_dram_parameter("b1s", [IN], F32, isOutput=False)   # 16*b1
    b2sd = nc.declare_dram_parameter("b2s", [D], F32, isOutput=False)   # 16*b2
    bed = nc.declare_dram_parameter("bes", [D], F32, isOutput=False)    # 16*be
    bxd = nc.declare_dram_parameter("bxs", [D], F32, isOutput=False)    # 16*bx
    byd = nc.declare_dram_parameter("bys", [D], F32, isOutput=False)    # 16*by
    # consts = [un_red_b*red_w0, red_w0/1024, red_w1/16384, 0]
    cst = nc.declare_dram_parameter("consts", [4], F32, isOutput=False)
    onesr16 = nc.declare_dram_parameter("onesr16", [1, 128], F16,
                                        isOutput=False)
    c16r = nc.declare_dram_parameter("c16r", [1, 128], F16, isOutput=False)
    b2rep = nc.declare_dram_parameter("b2rep", [4, D], F16, isOutput=False)
    out = nc.declare_dram_parameter("out", [bpc, D], F32, isOutput=True)

    with tile.TileContext(nc) as tc:
        _body(nc, tc, bpc, xt, w1p, w2hi, w2lo, wet, wxt, wyt, wr8d, ones8d,
              b1d, b2sd, bed, bxd, byd, cst, onesr16, c16r, b2rep, out)
    return nc


def _body(nc, tc, bpc, xt, w1p, w2hi, w2lo, wet, wxt, wyt, wr8d, ones8d,
          b1d, b2sd, bed, bxd, byd, cst, onesr16, c16r, b2rep, out):
    with (
        tc.tile_pool(name="wpool", bufs=1) as wpool,
        tc.tile_pool(name="xp", bufs=2) as xp,
        tc.tile_pool(name="h8p", bufs=2) as h8p,
        tc.tile_pool(name="u8p", bufs=2) as u8p,
        tc.tile_pool(name="xe8p", bufs=2) as xe8p,
        tc.tile_pool(name="ye8p", bufs=2) as ye8p,
        tc.tile_pool(name="e8p", bufs=1) as e8p,
        tc.tile_pool(name="sqp", bufs=1) as sqp,
        tc.tile_pool(name="tmpp", bufs=2) as tmpp,
        tc.tile_pool(name="rows", bufs=1) as rows,
        tc.tile_pool(name="rtmp", bufs=2) as rtmp,
        tc.tile_pool(name="bat", bufs=1) as bat,
        tc.tile_pool(name="bc16p", bufs=1) as bc16p,
        tc.tile_pool(name="mm2", bufs=2, space="PSUM") as mm2,
        tc.tile_pool(name="rpp", bufs=3, space="PSUM") as rpp,
        tc.tile_pool(name="bcp", bufs=1, space="PSUM") as bcp,
    ):
        # ---- persistent weights / constants ----
        # ordering: the m=0 fc1 block + batch-0 x8 go first so the PE can
        # start ~3us in; the rest of w1 streams m-major exactly in consume
        # order; w2lo (only used at the very end) goes last.
        b1_sb = wpool.tile([128, KI], F32)
        nc.sync.dma_start(b1_sb, b1d.rearrange("(o p) -> p o", p=128))
        onesr16_sb = wpool.tile([1, 128], F16)
        nc.sync.dma_start(onesr16_sb, onesr16[:, :])
        w1_sb = wpool.tile([128, KI, KIP, 2, 128], F8)
        nc.sync.dma_start(w1_sb[:, 0], w1p[0])
        # kp-chunked so fc1's first m-block can start on kp0 ~3us earlier;
        # on the ACT hwdge queue so x streams in parallel with w1 (sync q)
        first_x = xp.tile([128, KI, T], F8, tag="xt")
        for kp in range(KIP):
            nc.scalar.dma_start(first_x[:, 2 * kp:2 * kp + 2, :],
                                xt[0][:, 2 * kp:2 * kp + 2, :])
        for m in range(1, KI):
            nc.sync.dma_start(w1_sb[:, m], w1p[m])
        # HAM warm-up: ~5us of junk matmuls on the already-resident constant
        # row flips the PE clock gate to 8/8 while the real data streams in,
        # so the first fc1 blocks run at 2.4GHz instead of 1.2
        wup = mm2.tile([128, T], F32, tag="mm", name="wup")
        for _ in range(24):
            nc.tensor.matmul(wup[0:128, 0:128], onesr16_sb, onesr16_sb,
                             start=True, stop=True)
        # remaining weights stream behind w1 (emitted via an fc1(b0) hook so
        # coarsened semaphore waits can't gate fc1 on them)
        w2h_sb = wpool.tile([128, KIP, 2, D], F8)
        b2s_sb = wpool.tile([128, KD], F32)
        we_sb = wpool.tile([128, KDP, 2, D], F8)
        be_sb = wpool.tile([128, KD], F32)
        wr_sb = wpool.tile([128, KDP, 2, 16], F8)
        wx_sb = wpool.tile([128, KDP, 2, D], F8)
        bx_sb = wpool.tile([128, KD], F32)
        wy_sb = wpool.tile([128, KDP, 2, D], F8)
        by_sb = wpool.tile([128, KD], F32)
        ones_sb = wpool.tile([128, KDP, 2, 16], F8)
        c_sb = wpool.tile([1, 4], F32)
        c16r_sb = wpool.tile([1, 128], F16)
        b2r_sb = wpool.tile([4, D], F16)
        w2l_sb = wpool.tile([128, KIP, 2, D], F8)

        def emit_rest_weights():
            nc.sync.dma_start(w2h_sb, w2hi[:, :, :, :])
            nc.sync.dma_start(b2s_sb, b2sd.rearrange("(o p) -> p o", p=128))
            nc.sync.dma_start(we_sb, wet[:, :, :, :])
            nc.sync.dma_start(be_sb, bed.rearrange("(o p) -> p o", p=128))
            nc.sync.dma_start(wr_sb, wr8d[:, :, :, :])
            nc.sync.dma_start(wx_sb, wxt[:, :, :, :])
            nc.sync.dma_start(bx_sb, bxd.rearrange("(o p) -> p o", p=128))
            nc.sync.dma_start(wy_sb, wyt[:, :, :, :])
            nc.sync.dma_start(by_sb, byd.rearrange("(o p) -> p o", p=128))
            nc.sync.dma_start(ones_sb, ones8d[:, :, :, :])
            nc.sync.dma_start(c_sb, cst[None, :])
            nc.sync.dma_start(c16r_sb, c16r[:, :])
            nc.sync.dma_start(b2r_sb, b2rep[:, :])
            nc.sync.dma_start(w2l_sb, w2lo[:, :, :, :])

        # shared across batches: per-batch softmax 1/sum at partition b,
        # V (weighted h sums, real scale) and V/16 for the two-plane W2 mm
        smcol = bat.tile([4, 1], F32, tag="smcol", name="smcol")
        v16 = bat.tile([128, KI, 4], F16, tag="v16", name="v16")
        v16l = bat.tile([128, KI, 4], F16, tag="v16l", name="v16l")

        def alloc_batch(b):
            st = {"b": b}
            st["h8"] = h8p.tile([128, KI, T], F8, tag="h8", name=f"h8_{b}")
            st["xe8"] = xe8p.tile([128, KD, T], F8, tag="xe8", name=f"xe_{b}")
            st["ye8"] = ye8p.tile([128, KD, T], F8, tag="ye8", name=f"ye_{b}")
            st["invx"] = rows.tile([1, T], F32, tag="invx", name=f"ix_{b}")
            st["scores"] = rows.tile([1, T], F32, tag="scores", name=f"sc_{b}")
            return st

        def mm_pair(ps, w_slice, mv, kp, nkp):
            """One stationary block, two N=512 moving halves."""
            nc.tensor.matmul(ps[0], w_slice, mv[:, 2 * kp:2 * kp + 2, 0:NT],
                             start=(kp == 0), stop=(kp == nkp - 1),
                             perf_mode=DR)
            nc.tensor.matmul(ps[1], w_slice, mv[:, 2 * kp:2 * kp + 2, NT:T],
                             start=(kp == 0), stop=(kp == nkp - 1),
                             perf_mode=DR)

        def mm_pair2(ps, w_slice, mv, kp, nkp):
            """One stationary block, both halves of a [128, T] 2-bank psum."""
            mm_pair((ps[:, 0:NT], ps[:, NT:T]), w_slice, mv, kp, nkp)

        def fc1_part(st, hooks={}):
            x_sb = st["x_sb"]
            for m in range(KI):
                ps = mm2.tile([128, T], F32, tag="mm")
                for kp in range(KIP):
                    mm_pair2(ps, w1_sb[:, m, kp], x_sb, kp, KIP)
                # h8 = 16*relu(z): psum = 1024*z -> relu(psum/64 + 16*b1);
                # single ACT over both banks halves the evacuation cost
                nc.scalar.activation(st["h8"][:, m, :], ps,
                                     AF.Relu, bias=b1_sb[:, m:m + 1],
                                     scale=1.0 / SW)
                if m in hooks:
                    hooks[m]()

        def fc2_part(st, hooks={}):
            b = st["b"]
            u8 = u8p.tile([128, KD, T], F8, tag="u8", name=f"u8{b}")
            for m in range(KD):
                ps = mm2.tile([128, T], F32, tag="mm")
                for kp in range(KIP):
                    mm_pair2(ps, w2h_sb[:, kp, :, m * 128:(m + 1) * 128],
                            st["h8"], kp, KIP)
                # u8 = 16*u: psum = 1024*u -> psum/64 + 16*b2
                nc.scalar.activation(u8[:, m, :], ps, AF.Identity,
                                     bias=b2s_sb[:, m:m + 1], scale=1.0 / SW)
                if m in hooks:
                    hooks[m]()
            st["u8"] = u8

        def row_pair(rps, w_slice, mv, kp, nkp):
            """DR row matmul on both halves into [1, T] psum pair."""
            nc.tensor.matmul(rps[0][:, 0:NT], w_slice,
                             mv[:, 2 * kp:2 * kp + 2, 0:NT],
                             start=(kp == 0), stop=(kp == nkp - 1),
                             perf_mode=DR)
            nc.tensor.matmul(rps[1][:, 0:NT], w_slice,
                             mv[:, 2 * kp:2 * kp + 2, NT:T],
                             start=(kp == 0), stop=(kp == nkp - 1),
                             perf_mode=DR)

        def une_part(st, hooks={}):
            b = st["b"]
            u8 = st["u8"]
            e8 = e8p.tile([128, KD, T], F8, tag="e8", name=f"e8{b}")
            rps0 = rpp.tile([1, NT], F32, tag="row")
            rps1 = rpp.tile([1, NT], F32, tag="row")
            rps = (rps0, rps1)
            for m in range(KD):
                ps = mm2.tile([128, T], F32, tag="mm")
                for kp in range(KDP):
                    mm_pair2(ps, we_sb[:, kp, :, m * 128:(m + 1) * 128],
                             u8, kp, KDP)
                # e8 = 16*relu(e): psum = 1024*epre -> relu(psum/64 + 16*be)
                nc.scalar.activation(e8[:, m, :], ps, AF.Relu,
                                     bias=be_sb[:, m:m + 1], scale=1.0 / SW)
                if m in hooks:
                    hooks[m]()
                # wr rows trail one k-pair behind the une m-loop
                if m >= 3 and m % 2 == 1:
                    row_pair(rps, wr_sb[:, (m - 3) // 2, :, 0:1], e8,
                             (m - 3) // 2, KDP)
            row_pair(rps, wr_sb[:, KDP - 1, :, 0:1], e8, KDP - 1, KDP)
            # scores = rw0*un + rw0*br ; rps = 1024*un
            nc.scalar.activation(st["scores"][:, 0:NT], rps[0], AF.Identity,
                                 bias=c_sb[:, 0:1], scale=c_sb[:, 1:2])
            nc.scalar.activation(st["scores"][:, NT:T], rps[1], AF.Identity,
                                 bias=c_sb[:, 0:1], scale=c_sb[:, 1:2])

        def pw_part(st, which, hooks={}, defer=False):
            """pwx or pwy: embedding matmuls + fp8 squares + ss row sums.

            With defer=True the ||v||^2 rows are only DVE-copied to SBUF and
            the ln/exp norm chain runs later (x_norms/y_norms hooks inside
            the next batch's fc1, where the ACT queue has slack); inline the
            [1,512] ACT ops would delay the next phase's PSUM evacuations.
            """
            b = st["b"]
            u8 = st["u8"]
            if which == "x":
                w_sb, bias_sb, dst = wx_sb, bx_sb, st["xe8"]
            else:
                w_sb, bias_sb, dst = wy_sb, by_sb, st["ye8"]
            sq8 = sqp.tile([128, KD, T], F8, tag="sq", name=f"sq{which}{b}")
            rps0 = rpp.tile([1, NT], F32, tag="row")
            rps1 = rpp.tile([1, NT], F32, tag="row")
            rps = (rps0, rps1)
            for m in range(KD):
                ps = mm2.tile([128, T], F32, tag="mm")
                for kp in range(KDP):
                    mm_pair2(ps, w_sb[:, kp, :, m * 128:(m + 1) * 128],
                             u8, kp, KDP)
                # dst = 16*v: psum = 1024*v -> psum/64 + 16*bias
                nc.scalar.activation(dst[:, m, :], ps, AF.Identity,
                                     bias=bias_sb[:, m:m + 1], scale=1.0 / SW)
                # sq8 = v^2, alternating engines: DVE STT (dst/256)*dst and
                # ACT Square((dst/16)^2) split the ~1.2us/row cost so neither
                # queue becomes the phase bottleneck
                if m % 2 == 0:
                    nc.vector.scalar_tensor_tensor(
                        sq8[:, m, :], dst[:, m, :], 1.0 / 256.0, dst[:, m, :],
                        op0=ALU.mult, op1=ALU.mult)
                else:
                    nc.scalar.activation(sq8[:, m, :], dst[:, m, :],
                                         AF.Square, scale=1.0 / 16.0)
                if m in hooks:
                    hooks[m]()
                if m >= 3 and m % 2 == 1:
                    row_pair(rps, ones_sb[:, (m - 3) // 2, :, 0:1], sq8,
                             (m - 3) // 2, KDP)
            row_pair(rps, ones_sb[:, KDP - 1, :, 0:1], sq8, KDP - 1, KDP)
            # rps = ||v||^2 (real scale). rsqrt = exp(-0.5*ln(.)): Ln and Exp
            # live in ONE activation table set together with Relu/Identity,
            # so the ACT engine never thrashes ACT_TABLE_LOADs (Sqrt doesn't
            # share a set with Exp).
            if defer:
                ss = rtmp.tile([1, T], F32, tag="ss" + which)
                nc.vector.tensor_copy(ss[:, 0:NT], rps[0])
                nc.vector.tensor_copy(ss[:, NT:T], rps[1])
                st["ss" + which] = ss
            elif which == "x":
                # invx = rw1/16384 * 1/||xe||  (sign of rw1 kept in c_sb)
                for half in range(2):
                    ns = slice(half * NT, (half + 1) * NT)
                    lx = rtmp.tile([1, NT], F32, tag="rt")
                    nc.scalar.activation(lx, rps[half], AF.Ln)
                    t0 = rtmp.tile([1, NT], F32, tag="rt")
                    nc.scalar.activation(t0, lx, AF.Exp, scale=-0.5)
                    nc.vector.tensor_scalar_mul(st["invx"][:, ns], t0,
                                                c_sb[:, 2:3])
            else:
                # t1h = 1/(16||ye||) = exp(-0.5*ln(ssy) - ln(16))
                for half in range(2):
                    ly = rtmp.tile([1, NT], F32, tag="rt")
                    nc.scalar.activation(ly, rps[half], AF.Ln)
                    t1 = rtmp.tile([1, NT], F32, tag="rt")
                    nc.scalar.activation(t1, ly, AF.Exp, scale=-0.5,
                                         bias=c_sb[:, 3:4])
                    t1h = rtmp.tile([1, NT], F16, tag="rth")
                    nc.vector.tensor_copy(t1h, t1)
                    st["t1h_%d" % half] = t1h

        def x_norms(st):
            # deferred: invx = rw1/16384 * 1/||xe|| over the full [1, T] row
            lx = rtmp.tile([1, T], F32, tag="rtw")
            nc.scalar.activation(lx, st["ssx"], AF.Ln)
            t0 = rtmp.tile([1, T], F32, tag="rtw")
            nc.scalar.activation(t0, lx, AF.Exp, scale=-0.5)
            nc.vector.tensor_scalar_mul(st["invx"], t0, c_sb[:, 2:3])

        def y_norms(st):
            # deferred: t1h = 1/(16||ye||) over the full [1, T] row
            ly = rtmp.tile([1, T], F32, tag="rtw")
            nc.scalar.activation(ly, st["ssy"], AF.Ln)
            t1 = rtmp.tile([1, T], F32, tag="rtw")
            nc.scalar.activation(t1, ly, AF.Exp, scale=-0.5,
                                 bias=c_sb[:, 3:4])
            t1h = rtmp.tile([1, T], F16, tag="rthw")
            nc.vector.tensor_copy(t1h, t1)
            st["t1h_0"] = t1h[:, 0:NT]
            st["t1h_1"] = t1h[:, NT:T]

        def y_pe(st):
            # yn = ye8 * (1/(16||ye||)) broadcast; full-row STT accumulates
            # straight into the ybar sum (no per-half partials)
            b = st["b"]
            ivb16 = bc16p.tile([128, T], F16, tag="bc16")
            for ti in range(NTT):
                ns = slice(ti * NT, (ti + 1) * NT)
                ivb = bcp.tile([128, NT], F32, tag="bc")
                nc.tensor.matmul(ivb, onesr16_sb, st["t1h_%d" % ti],
                                 start=True, stop=True)
                # DVE copy: keeps the ACT queue free for PSUM evacuations
                nc.vector.tensor_copy(ivb16[:, ns], ivb)
            ybf = bat.tile([128, KDP, 2, 1], F32, tag="ybf", name=f"yf{b}")
            for m in range(KD):
                tmp = tmpp.tile([128, T], F16, tag="tmp")
                nc.vector.scalar_tensor_tensor(
                    tmp, st["ye8"][:, m, :], 1.0, ivb16,
                    op0=ALU.mult, op1=ALU.mult,
                    accum_out=ybf[:, m // 2, m % 2, :])
            # padded [.., 2, 16] fp8: dual-row ldweights needs the k-pair
            # step 16B-aligned
            ybar8 = bat.tile([128, KDP, 2, 16], F8, tag="ybar",
                             name=f"yb{b}")
            nc.vector.tensor_copy(ybar8[:, :, :, 0:1], ybf)
            st["ybar8"] = ybar8

        def q_scores(st):
            # q = 256*(xe.ybar) ; scores += q * invx  (consts folded)
            b = st["b"]
            mxp = rows.tile([1, NTT], F32, tag="mxp", name=f"mxp{b}")
            for ti in range(NTT):
                ns = slice(ti * NT, (ti + 1) * NT)
                qps = rpp.tile([1, NT], F32, tag="row")
                for kp in range(KDP):
                    nc.tensor.matmul(qps,
                                     st["ybar8"][:, kp, :, 0:1],
                                     st["xe8"][:, 2 * kp:2 * kp + 2, ns],
                                     start=(kp == 0), stop=(kp == KDP - 1),
                                     perf_mode=DR)
                s0 = rtmp.tile([1, NT], F32, tag="rt")
                nc.vector.tensor_mul(s0, qps, st["invx"][:, ns])
                nc.vector.tensor_add(st["scores"][:, ns], st["scores"][:, ns],
                                     s0)
                nc.vector.reduce_max(mxp[:, ti:ti + 1], st["scores"][:, ns],
                                     axis=AX.X)
            mx = rows.tile([1, 1], F32, tag="mx", name=f"mx{b}")
            nc.vector.reduce_max(mx, mxp, axis=AX.X, negate=True)
            st["mx"] = mx

        def q_exp(st):
            # emitted a few m-blocks after q_scores so the exp's deps are
            # long resolved when the in-order ACT queue reaches it; exp
            # writes the fp16 weights row directly (no extra copy)
            b = st["b"]
            ewh = rows.tile([1, T], F16, tag="ewh", name=f"ew{b}")
            nc.scalar.activation(ewh, st["scores"], AF.Exp, bias=st["mx"])
            st["ewh"] = ewh
            # 1/(64*sum) lands at partition b of smcol (per-partition ACT
            # scale on the final [4, D] correction matmul; 1/SW pre-folded)
            sm = rows.tile([1, 1], F32, tag="sm", name=f"sm{b}")
            nc.vector.reduce_sum(sm, ewh, axis=AX.X)
            nc.vector.tensor_scalar_mul(sm, sm, SW)
            smi = rows.tile([1, 1], F32, tag="smi", name=f"smi{b}")
            nc.vector.reciprocal(smi, sm)
            nc.sync.dma_start(smcol[b:b + 1, :], smi)

        def q_rows(st):
            # tail-only: q contribution to a separate row, BEFORE une has
            # produced the un-part of scores (the DVE muls then hide under
            # une's matmuls)
            b = st["b"]
            qrow = rows.tile([1, T], F32, tag="qrow", name=f"qr{b}")
            for ti in range(NTT):
                ns = slice(ti * NT, (ti + 1) * NT)
                qps = rpp.tile([1, NT], F32, tag="row")
                for kp in range(KDP):
                    nc.tensor.matmul(qps,
                                     st["ybar8"][:, kp, :, 0:1],
                                     st["xe8"][:, 2 * kp:2 * kp + 2, ns],
                                     start=(kp == 0), stop=(kp == KDP - 1),
                                     perf_mode=DR)
                nc.vector.tensor_mul(qrow[:, ns], qps, st["invx"][:, ns])
            st["qrow"] = qrow

        def scores_fin(st):
            b = st["b"]
            nc.vector.tensor_add(st["scores"], st["scores"], st["qrow"])
            mx = rows.tile([1, 1], F32, tag="mx", name=f"mx{b}")
            nc.vector.reduce_max(mx, st["scores"], axis=AX.X, negate=True)
            st["mx"] = mx

        def pass2_w(st, tail=False):
            # V[:, :, b] = sum_t w[t]*h[t]: bcast ew/16 (c16r stationary) then
            # fused DVE multiply+accumulate over h8 = 16*h -> real-scale V.
            # Both halves merged into one [128, T] STT per m (free-axis accum
            # covers the full token range directly).
            b = st["b"]
            wbc16 = bc16p.tile([128, T], F16, tag="bc16w", name=f"wb{b}")
            for ti in range(NTT):
                ns = slice(ti * NT, (ti + 1) * NT)
                wbc = bcp.tile([128, NT], F32, tag="bc")
                nc.tensor.matmul(wbc, c16r_sb, st["ewh"][:, ns],
                                 start=True, stop=True)
                nc.vector.tensor_copy(wbc16[:, ns], wbc)
            vacc = bat.tile([128, KI, 1], F32, tag="vacc", name=f"va{b}")
            for m in range(KI):
                tmp = tmpp.tile([128, T], F16, tag="tmp")
                nc.vector.scalar_tensor_tensor(
                    tmp, st["h8"][:, m, :], 1.0, wbc16,
                    op0=ALU.mult, op1=ALU.mult,
                    accum_out=vacc[:, m, :])
                if tail:
                    # per-k V write unblocks correction matmul k
                    nc.vector.tensor_copy(v16[:, m, b:b + 1], vacc[:, m, :])
                    nc.vector.tensor_scalar_mul(v16l[:, m, b:b + 1],
                                                v16[:, m, b:b + 1],
                                                1.0 / 16.0)
            if not tail:
                nc.vector.tensor_copy(v16[:, :, b:b + 1], vacc)
                nc.vector.tensor_scalar_mul(v16l[:, :, b:b + 1],
                                            v16[:, :, b:b + 1], 1.0 / 16.0)

        def final_correction():
            # out[b, :] = (V[:, b] @ (W2hi + W2lo/16)) / (64*sum_b) + b2
            # k-outer: correction matmuls for k stream as soon as the per-k
            # V writes land (two separate banks, one accumulation region each)
            HD = D // 2
            psc_a = mm2.tile([128, T], F32, tag="mm", name="pc0")
            psc_b = mm2.tile([128, T], F32, tag="mm", name="pc1")
            psc = [psc_a[0:4, 0:HD], psc_b[0:4, 0:HD]]
            for kp in range(KIP):
                for j in range(2):
                    k = 2 * kp + j
                    for h in range(2):
                        hs = slice(h * HD, (h + 1) * HD)
                        nc.tensor.matmul(psc[h], v16[:, k, :],
                                         w2h_sb[:, kp, j, hs],
                                         start=(kp == 0 and j == 0),
                                         stop=False)
                        nc.tensor.matmul(psc[h], v16l[:, k, :],
                                         w2l_sb[:, kp, j, hs],
                                         start=False,
                                         stop=(kp == KIP - 1 and j == 1))
            outf = bat.tile([4, D], F32, tag="outf", name="outf")
            for h in range(2):
                hs = slice(h * HD, (h + 1) * HD)
                nc.scalar.activation(outf[:, hs], psc[h], AF.Identity,
                                     scale=smcol)
                nc.vector.tensor_add(outf[:, hs], outf[:, hs], b2r_sb[:, hs])
            nc.sync.dma_start(out[:, :], outf)

        def prefetch_x(b):
            if b >= bpc:
                return {}
            st_x = xp.tile([128, KI, T], F8, tag="xt", name=f"xt{b}")
            nc.scalar.dma_start(st_x, xt[b])
            return st_x

        def mk(f, *a):
            return lambda: f(*a)

        prev = None
        next_x = first_x
        for b in range(bpc):
            st = alloc_batch(b)
            st["x_sb"] = next_x
            holder = {}
            # fc1 hooks: prev batch's softmax chain + the next x prefetch
            # hide under the 12 dense m-blocks
            todo = {}

            def add_hook(m, f):
                todo.setdefault(m, []).append(f)

            if b == 0:
                add_hook(1, emit_rest_weights)
            if b + 1 < bpc:
                add_hook(0, lambda bb=b + 1: holder.__setitem__(
                    "x", prefetch_x(bb)))
            if prev is not None:
                # chore train packed early so pass2_w's DVE grind completes
                # during fc2 and leaves the DVE free for the next pw phases
                add_hook(0, mk(y_norms, prev))
                add_hook(1, mk(x_norms, prev))
                add_hook(3, mk(y_pe, prev))
                add_hook(7, mk(q_scores, prev))
                add_hook(9, mk(q_exp, prev))
                add_hook(10, mk(pass2_w, prev))
            hooks = {m: (lambda fs=fs: [f() for f in fs])
                     for m, fs in todo.items()}
            fc1_part(st, hooks)
            fc2_part(st)
            if b < bpc - 1:
                une_part(st)
                pw_part(st, "x", defer=True)
                pw_part(st, "y", defer=True)
            else:
                # last batch: y_pe chores hook into pwx's matmul stream, une
                # goes LAST so the q/invx chain hides under its matmuls;
                # only the exp chain, weighted sum and final correction
                # remain exposed
                pw_part(st, "y")
                pw_part(st, "x", hooks={2: mk(y_pe, st)})
                q_rows(st)
                une_part(st)
            next_x = holder.get("x")
            prev = st
        scores_fin(prev)
        q_exp(prev)
        pass2_w(prev, tail=True)
        final_correction()


_CACHE = {}


def _get_nc():
    if "nc" not in _CACHE:
        nc = build_nc(BPC)
        nc.finalize()
        _CACHE["nc"] = nc
    return _CACHE["nc"]


def _q8(a, scale):
    return (np.asarray(a, np.float32) * scale).astype(ml_dtypes.float8_e4m3)


def _pack_dual(w8):
    """[K, M] fp8 -> [128, K/256, 2, M] dual-row ldweights layout."""
    K, M = w8.shape
    kp = K // 256
    return np.ascontiguousarray(w8.reshape(kp, 2, 128, M).transpose(2, 0, 1, 3))


def _pack_w1(w18):
    """[K=IN, M=IN] fp8 -> m-major [12, 128, 6, 2, 128]."""
    a = w18.reshape(KIP, 2, 128, KI, 128)        # kp, q, p, mo, mi
    return np.ascontiguousarray(a.transpose(3, 2, 0, 1, 4))


def _pack_row(v8):
    """[K] fp8 -> [128, K/256, 2, 16] padded dual layout, value at col 0."""
    K = v8.shape[0]
    kp = K // 256
    outv = np.zeros([128, kp, 2, 16], ml_dtypes.float8_e4m3)
    outv[:, :, :, 0] = v8.reshape(kp, 2, 128).transpose(2, 0, 1)
    return outv


def make_in_maps(x, fc1_w, fc1_b, fc2_w, fc2_b, un_emb_w, un_emb_b,
                 un_red_w, un_red_b, pw_x_w, pw_x_b, pw_y_w, pw_y_b, red_w):
    w2s = np.ascontiguousarray(fc2_w.T).astype(np.float32) * SW
    w2hi = w2s.astype(ml_dtypes.float8_e4m3)
    w2lo = ((w2s - w2hi.astype(np.float32)) * 16.0).astype(
        ml_dtypes.float8_e4m3)
    shared = {
        "w1p": _pack_w1(_q8(np.ascontiguousarray(fc1_w.T), SW)),
        "w2hi": _pack_dual(w2hi),
        "w2lo": _pack_dual(w2lo),
        "wet": _pack_dual(_q8(np.ascontiguousarray(un_emb_w.T), SW)),
        "wxt": _pack_dual(_q8(np.ascontiguousarray(pw_x_w.T), SW)),
        "wyt": _pack_dual(_q8(np.ascontiguousarray(pw_y_w.T), SW)),
        "wr8": _pack_row(_q8(un_red_w[0], SW)),
        "ones8": _pack_row(np.ones([D], np.float32).astype(
            ml_dtypes.float8_e4m3)),
        "b1s": np.asarray(fc1_b, np.float32) * SX,
        "b2s": np.asarray(fc2_b, np.float32) * SX,
        "bes": np.asarray(un_emb_b, np.float32) * SX,
        "bxs": np.asarray(pw_x_b, np.float32) * SX,
        "bys": np.asarray(pw_y_b, np.float32) * SX,
        "consts": np.array([un_red_b[0] * red_w[0], red_w[0] / 1024.0,
                            red_w[1] / 16384.0, -np.log(16.0)], np.float32),
        "onesr16": np.ones([1, 128], np.float16),
        "c16r": np.full([1, 128], 1.0 / 16.0, np.float16),
        "b2rep": np.tile(np.asarray(fc2_b, np.float16)[None, :], (4, 1)),
    }
    in_maps = []
    for c in range(NCORES):
        a = _q8(x[c * BPC:(c + 1) * BPC], SX)          # [bpc, T, IN] fp8
        a = a.reshape(BPC, T, KI, 128).transpose(0, 3, 2, 1)
        in_maps.append({"xt": np.ascontiguousarray(a), **shared})
    return in_maps


def kernel(**inputs) -> np.ndarray:
    inputs = {k: np.asarray(v) for k, v in inputs.items()}
    nc = _get_nc()
    in_maps = make_in_maps(**inputs)
    res = run_bass_kernel_spmd(nc, in_maps, core_ids=list(range(NCORES)))
    return np.concatenate([res.results[c]["out"] for c in range(NCORES)],
                          axis=0)


# revision 56
# speedup vs baseline: 1.0078x; 1.0078x over previous
"""Trainium2 Bass kernel for FGAEmbedder (B=32, T=1024, IN=1536, D=768).

Math (identical to the reference up to float reassociation + fp8 noise;
validated vs the jax reference in numpy at rel_err ~1.1e-2 < 2e-2):
    h  = relu(x @ W1^T + b1)           [B,T,IN]   fp8 (x8/w1 fp8, DR)
    u  = h @ W2^T + b2                 [B,T,D]    fp8
    e  = relu(u @ We^T + be)  ; un = e @ Wr^T + br       (score path)
    xe = u @ Wx^T + bx ; ye = u @ Wy^T + by              (score path)
    pw[t] = (xe[t] . ybar) / ||xe[t]||, ybar = sum_s ye[s]/||ye[s]||
    w  = softmax(rw0*un + rw1*pw)
    out = (sum_t w[t] * h[t]) @ W2^T + b2        <- fc2 is linear, so the
          weighted sum is pushed through W2 (two-plane fp8 hi+lo weights).

Perf structure:
  - every big matmul is fp8 DoubleRow, and each stationary weight block is
    loaded ONCE and used for TWO N=512 moving halves (the redundant second
    LDWEIGHTS of an identical stationary hides completely under the matmul,
    ~213ns/MM vs ~300ns when alternating LDW/MM with distinct weights).
  - fc1's ACT evacuation writes h8 (16*h fp8) directly: no fp16 h tensor, no
    DVE quantize pass; the exact-path weighted sum V runs off h8.
  - row reductions (un-reduce, sum-of-squares for the cosine norms) are fp8
    DoubleRow matmuls with 16B-padded dual stationaries.
  - w1 is packed m-major on the host so the fc1 weight stream arrives in
    exactly the order the PE consumes it; x8 is prefetched one batch ahead.

Sharding: data-parallel over batch, 4 batches per core, no collectives.
"""

import numpy as np
import ml_dtypes

import concourse.bass as bass
import concourse.bacc as bacc
import concourse.mybir as mybir
import concourse.tile as tile
from concourse.bass_utils import run_bass_kernel_spmd

# The kernel's ACT functions are Relu/Identity/Copy/Ln/Exp. Ln and Exp only
# coexist in the "natural_log_exp_and_others" table set, but the greedy set
# chooser maps exp->"exp_and_others" and ln->"natural_log", thrashing
# ACT_TABLE_LOADs (1.5us each) all kernel long. Filter Exp/Ln out of every
# other set (names and order — and therefore act_func_set_ids — unchanged)
# so the chooser lands on the combined set: exactly one load at startup.
_orig_gat = bacc.get_activation_tables


def _gat_single_set(arch):
    tables = _orig_gat(arch)
    AFt = mybir.ActivationFunctionType
    out = {}
    for name, fns in tables.items():
        fns = set(fns)
        if name != "natural_log_exp_and_others":
            fns.discard(AFt.Exp)
            fns.discard(AFt.Ln)
        out[name] = fns
    return out


bacc.get_activation_tables = _gat_single_set

B, T, IN, D = 32, 1024, 1536, 768
NCORES = 8
BPC = B // NCORES        # batches per core
NT = 512                 # matmul moving free dim (one PSUM bank)
NTT = T // NT            # moving halves
KI = IN // 128           # 12 feature tiles of the 1536 dim
KIP = KI // 2            # 6 fp8 double-row k-pairs
KD = D // 128            # 6 feature tiles of the 768 dim
KDP = KD // 2            # 3 fp8 double-row k-pairs

SX = 16.0                # fp8 activation scale
SW = 64.0                # fp8 weight scale

F16 = mybir.dt.float16
F8 = mybir.dt.float8e4
F32 = mybir.dt.float32
AF = mybir.ActivationFunctionType
ALU = mybir.AluOpType
AX = mybir.AxisListType
DR = mybir.MatmulPerfMode.DoubleRow


def build_nc(bpc: int = BPC) -> bass.Bass:
    nc = bacc.Bacc()

    # x8 pre-packed partition-major on the host: [b, p, ko, t] so the per-
    # batch DMA is one fully-contiguous 1.5MB transfer (a (ko p) t gather in
    # 1KB chunks only sustains ~140GB/s and starved the first fc1)
    xt = nc.declare_dram_parameter("xt", [bpc, 128, KI, T], F8, isOutput=False)
    # w1 m-major dual pack: [m, 128, kp, 2, 128]
    w1p = nc.declare_dram_parameter("w1p", [KI, 128, KIP, 2, 128], F8,
                                    isOutput=False)
    w2hi = nc.declare_dram_parameter("w2hi", [128, KIP, 2, D], F8,
                                     isOutput=False)
    w2lo = nc.declare_dram_parameter("w2lo", [128, KIP, 2, D], F8,
                                     isOutput=False)
    wet = nc.declare_dram_parameter("wet", [128, KDP, 2, D], F8,
                                    isOutput=False)
    wxt = nc.declare_dram_parameter("wxt", [128, KDP, 2, D], F8,
                                    isOutput=False)
    wyt = nc.declare_dram_parameter("wyt", [128, KDP, 2, D], F8,
                                    isOutput=False)
    wr8d = nc.declare_dram_parameter("wr8", [128, KDP, 2, 16], F8,
                                     isOutput=False)
    ones8d = nc.declare_dram_parameter("ones8", [128, KDP, 2, 16], F8,
                                       isOutput=False)
    b1d = nc.declare_dram_parameter("b1s", [IN], F32, isOutput=False)   # 16*b1
    b2sd = nc.declare_dram_parameter("b2s", [D], F32, isOutput=False)   # 16*b2
    bed = nc.declare_dram_parameter("bes", [D], F32, isOutput=False)    # 16*be
    bxd = nc.declare_dram_parameter("bxs", [D], F32, isOutput=False)    # 16*bx
    byd = nc.declare_dram_parameter("bys", [D], F32, isOutput=False)    # 16*by
    # consts = [un_red_b*red_w0, red_w0/1024, red_w1/16384, 0]
    cst = nc.declare_dram_parameter("consts", [4], F32, isOutput=False)
    onesr16 = nc.declare_dram_parameter("onesr16", [1, 128], F16,
                                        isOutput=False)
    c16r = nc.declare_dram_parameter("c16r", [1, 128], F16, isOutput=False)
    b2rep = nc.declare_dram_parameter("b2rep", [4, D], F16, isOutput=False)
    out = nc.declare_dram_parameter("out", [bpc, D], F32, isOutput=True)

    with tile.TileContext(nc) as tc:
        _body(nc, tc, bpc, xt, w1p, w2hi, w2lo, wet, wxt, wyt, wr8d, ones8d,
              b1d, b2sd, bed, bxd, byd, cst, onesr16, c16r, b2rep, out)
    return nc


def _body(nc, tc, bpc, xt, w1p, w2hi, w2lo, wet, wxt, wyt, wr8d, ones8d,
          b1d, b2sd, bed, bxd, byd, cst, onesr16, c16r, b2rep, out):
    with (
        tc.tile_pool(name="wpool", bufs=1) as wpool,
        tc.tile_pool(name="xp", bufs=2) as xp,
        tc.tile_pool(name="h8p", bufs=2) as h8p,
        tc.tile_pool(name="u8p", bufs=2) as u8p,
        tc.tile_pool(name="xe8p", bufs=2) as xe8p,
        tc.tile_pool(name="ye8p", bufs=2) as ye8p,
        tc.tile_pool(name="e8p", bufs=1) as e8p,
        tc.tile_pool(name="sqp", bufs=1) as sqp,
        tc.tile_pool(name="tmpp", bufs=2) as tmpp,
        tc.tile_pool(name="rows", bufs=1) as rows,
        tc.tile_pool(name="rtmp", bufs=2) as rtmp,
        tc.tile_pool(name="bat", bufs=1) as bat,
        tc.tile_pool(name="bc16p", bufs=1) as bc16p,
        tc.tile_pool(name="mm2", bufs=2, space="PSUM") as mm2,
        tc.tile_pool(name="rpp", bufs=3, space="PSUM") as rpp,
        tc.tile_pool(name="bcp", bufs=1, space="PSUM") as bcp,
    ):
        # ---- persistent weights / constants ----
        # ordering: the m=0 fc1 block + batch-0 x8 go first so the PE can
        # start ~3us in; the rest of w1 streams m-major exactly in consume
        # order; w2lo (only used at the very end) goes last.
        b1_sb = wpool.tile([128, KI], F32)
        nc.sync.dma_start(b1_sb, b1d.rearrange("(o p) -> p o", p=128))
        onesr16_sb = wpool.tile([1, 128], F16)
        nc.sync.dma_start(onesr16_sb, onesr16[:, :])
        w1_sb = wpool.tile([128, KI, KIP, 2, 128], F8)
        nc.sync.dma_start(w1_sb[:, 0], w1p[0])
        # kp-chunked so fc1's first m-block can start on kp0 ~3us earlier;
        # on the ACT hwdge queue so x streams in parallel with w1 (sync q)
        first_x = xp.tile([128, KI, T], F8, tag="xt")
        for kp in range(KIP):
            nc.scalar.dma_start(first_x[:, 2 * kp:2 * kp + 2, :],
                                xt[0][:, 2 * kp:2 * kp + 2, :])
        for m in range(1, KI):
            nc.sync.dma_start(w1_sb[:, m], w1p[m])
        w2h_sb = wpool.tile([128, KIP, 2, D], F8)
        nc.sync.dma_start(w2h_sb, w2hi[:, :, :, :])
        b2s_sb = wpool.tile([128, KD], F32)
        nc.sync.dma_start(b2s_sb, b2sd.rearrange("(o p) -> p o", p=128))
        we_sb = wpool.tile([128, KDP, 2, D], F8)
        nc.sync.dma_start(we_sb, wet[:, :, :, :])
        be_sb = wpool.tile([128, KD], F32)
        nc.sync.dma_start(be_sb, bed.rearrange("(o p) -> p o", p=128))
        wr_sb = wpool.tile([128, KDP, 2, 16], F8)
        nc.sync.dma_start(wr_sb, wr8d[:, :, :, :])
        wx_sb = wpool.tile([128, KDP, 2, D], F8)
        nc.sync.dma_start(wx_sb, wxt[:, :, :, :])
        bx_sb = wpool.tile([128, KD], F32)
        nc.sync.dma_start(bx_sb, bxd.rearrange("(o p) -> p o", p=128))
        wy_sb = wpool.tile([128, KDP, 2, D], F8)
        nc.sync.dma_start(wy_sb, wyt[:, :, :, :])
        by_sb = wpool.tile([128, KD], F32)
        nc.sync.dma_start(by_sb, byd.rearrange("(o p) -> p o", p=128))
        ones_sb = wpool.tile([128, KDP, 2, 16], F8)
        nc.sync.dma_start(ones_sb, ones8d[:, :, :, :])
        c_sb = wpool.tile([1, 4], F32)
        nc.sync.dma_start(c_sb, cst[None, :])
        c16r_sb = wpool.tile([1, 128], F16)
        nc.sync.dma_start(c16r_sb, c16r[:, :])
        b2r_sb = wpool.tile([4, D], F16)
        nc.sync.dma_start(b2r_sb, b2rep[:, :])
        w2l_sb = wpool.tile([128, KIP, 2, D], F8)
        nc.sync.dma_start(w2l_sb, w2lo[:, :, :, :])

        # shared across batches: per-batch softmax 1/sum at partition b,
        # V (weighted h sums, real scale) and V/16 for the two-plane W2 mm
        smcol = bat.tile([4, 1], F32, tag="smcol", name="smcol")
        v16 = bat.tile([128, KI, 4], F16, tag="v16", name="v16")
        v16l = bat.tile([128, KI, 4], F16, tag="v16l", name="v16l")

        def alloc_batch(b):
            st = {"b": b}
            st["h8"] = h8p.tile([128, KI, T], F8, tag="h8", name=f"h8_{b}")
            st["xe8"] = xe8p.tile([128, KD, T], F8, tag="xe8", name=f"xe_{b}")
            st["ye8"] = ye8p.tile([128, KD, T], F8, tag="ye8", name=f"ye_{b}")
            st["invx"] = rows.tile([1, T], F32, tag="invx", name=f"ix_{b}")
            st["scores"] = rows.tile([1, T], F32, tag="scores", name=f"sc_{b}")
            return st

        def mm_pair(ps, w_slice, mv, kp, nkp):
            """One stationary block, two N=512 moving halves."""
            nc.tensor.matmul(ps[0], w_slice, mv[:, 2 * kp:2 * kp + 2, 0:NT],
                             start=(kp == 0), stop=(kp == nkp - 1),
                             perf_mode=DR)
            nc.tensor.matmul(ps[1], w_slice, mv[:, 2 * kp:2 * kp + 2, NT:T],
                             start=(kp == 0), stop=(kp == nkp - 1),
                             perf_mode=DR)

        def mm_pair2(ps, w_slice, mv, kp, nkp):
            """One stationary block, both halves of a [128, T] 2-bank psum."""
            mm_pair((ps[:, 0:NT], ps[:, NT:T]), w_slice, mv, kp, nkp)

        def fc1_part(st, hooks={}):
            x_sb = st["x_sb"]
            for m in range(KI):
                ps = mm2.tile([128, T], F32, tag="mm")
                for kp in range(KIP):
                    mm_pair2(ps, w1_sb[:, m, kp], x_sb, kp, KIP)
                # h8 = 16*relu(z): psum = 1024*z -> relu(psum/64 + 16*b1);
                # single ACT over both banks halves the evacuation cost
                nc.scalar.activation(st["h8"][:, m, :], ps,
                                     AF.Relu, bias=b1_sb[:, m:m + 1],
                                     scale=1.0 / SW)
                if m in hooks:
                    hooks[m]()

        def fc2_part(st, hooks={}):
            b = st["b"]
            u8 = u8p.tile([128, KD, T], F8, tag="u8", name=f"u8{b}")
            for m in range(KD):
                ps = mm2.tile([128, T], F32, tag="mm")
                for kp in range(KIP):
                    mm_pair2(ps, w2h_sb[:, kp, :, m * 128:(m + 1) * 128],
                            st["h8"], kp, KIP)
                # u8 = 16*u: psum = 1024*u -> psum/64 + 16*b2
                nc.scalar.activation(u8[:, m, :], ps, AF.Identity,
                                     bias=b2s_sb[:, m:m + 1], scale=1.0 / SW)
                if m in hooks:
                    hooks[m]()
            st["u8"] = u8

        def row_pair(rps, w_slice, mv, kp, nkp):
            """DR row matmul on both halves into [1, T] psum pair."""
            nc.tensor.matmul(rps[0][:, 0:NT], w_slice,
                             mv[:, 2 * kp:2 * kp + 2, 0:NT],
                             start=(kp == 0), stop=(kp == nkp - 1),
                             perf_mode=DR)
            nc.tensor.matmul(rps[1][:, 0:NT], w_slice,
                             mv[:, 2 * kp:2 * kp + 2, NT:T],
                             start=(kp == 0), stop=(kp == nkp - 1),
                             perf_mode=DR)

        def une_part(st, hooks={}):
            b = st["b"]
            u8 = st["u8"]
            e8 = e8p.tile([128, KD, T], F8, tag="e8", name=f"e8{b}")
            rps0 = rpp.tile([1, NT], F32, tag="row")
            rps1 = rpp.tile([1, NT], F32, tag="row")
            rps = (rps0, rps1)
            for m in range(KD):
                ps = mm2.tile([128, T], F32, tag="mm")
                for kp in range(KDP):
                    mm_pair2(ps, we_sb[:, kp, :, m * 128:(m + 1) * 128],
                             u8, kp, KDP)
                # e8 = 16*relu(e): psum = 1024*epre -> relu(psum/64 + 16*be)
                nc.scalar.activation(e8[:, m, :], ps, AF.Relu,
                                     bias=be_sb[:, m:m + 1], scale=1.0 / SW)
                if m in hooks:
                    hooks[m]()
                # wr rows trail one k-pair behind the une m-loop
                if m >= 3 and m % 2 == 1:
                    row_pair(rps, wr_sb[:, (m - 3) // 2, :, 0:1], e8,
                             (m - 3) // 2, KDP)
            row_pair(rps, wr_sb[:, KDP - 1, :, 0:1], e8, KDP - 1, KDP)
            # scores = rw0*un + rw0*br ; rps = 1024*un
            nc.scalar.activation(st["scores"][:, 0:NT], rps[0], AF.Identity,
                                 bias=c_sb[:, 0:1], scale=c_sb[:, 1:2])
            nc.scalar.activation(st["scores"][:, NT:T], rps[1], AF.Identity,
                                 bias=c_sb[:, 0:1], scale=c_sb[:, 1:2])

        def pw_part(st, which, hooks={}, defer=False):
            """pwx or pwy: embedding matmuls + fp8 squares + ss row sums.

            With defer=True the ||v||^2 rows are only DVE-copied to SBUF and
            the ln/exp norm chain runs later (x_norms/y_norms hooks inside
            the next batch's fc1, where the ACT queue has slack); inline the
            [1,512] ACT ops would delay the next phase's PSUM evacuations.
            """
            b = st["b"]
            u8 = st["u8"]
            if which == "x":
                w_sb, bias_sb, dst = wx_sb, bx_sb, st["xe8"]
            else:
                w_sb, bias_sb, dst = wy_sb, by_sb, st["ye8"]
            sq8 = sqp.tile([128, KD, T], F8, tag="sq", name=f"sq{which}{b}")
            rps0 = rpp.tile([1, NT], F32, tag="row")
            rps1 = rpp.tile([1, NT], F32, tag="row")
            rps = (rps0, rps1)
            for m in range(KD):
                ps = mm2.tile([128, T], F32, tag="mm")
                for kp in range(KDP):
                    mm_pair2(ps, w_sb[:, kp, :, m * 128:(m + 1) * 128],
                             u8, kp, KDP)
                # dst = 16*v: psum = 1024*v -> psum/64 + 16*bias
                nc.scalar.activation(dst[:, m, :], ps, AF.Identity,
                                     bias=bias_sb[:, m:m + 1], scale=1.0 / SW)
                # sq8 = v^2, alternating engines: DVE STT (dst/256)*dst and
                # ACT Square((dst/16)^2) split the ~1.2us/row cost so neither
                # queue becomes the phase bottleneck
                if m % 2 == 0:
                    nc.vector.scalar_tensor_tensor(
                        sq8[:, m, :], dst[:, m, :], 1.0 / 256.0, dst[:, m, :],
                        op0=ALU.mult, op1=ALU.mult)
                else:
                    nc.scalar.activation(sq8[:, m, :], dst[:, m, :],
                                         AF.Square, scale=1.0 / 16.0)
                if m in hooks:
                    hooks[m]()
                if m >= 3 and m % 2 == 1:
                    row_pair(rps, ones_sb[:, (m - 3) // 2, :, 0:1], sq8,
                             (m - 3) // 2, KDP)
            row_pair(rps, ones_sb[:, KDP - 1, :, 0:1], sq8, KDP - 1, KDP)
            # rps = ||v||^2 (real scale). rsqrt = exp(-0.5*ln(.)): Ln and Exp
            # live in ONE activation table set together with Relu/Identity,
            # so the ACT engine never thrashes ACT_TABLE_LOADs (Sqrt doesn't
            # share a set with Exp).
            if defer:
                ss = rtmp.tile([1, T], F32, tag="ss" + which)
                nc.vector.tensor_copy(ss[:, 0:NT], rps[0])
                nc.vector.tensor_copy(ss[:, NT:T], rps[1])
                st["ss" + which] = ss
            elif which == "x":
                # invx = rw1/16384 * 1/||xe||  (sign of rw1 kept in c_sb)
                for half in range(2):
                    ns = slice(half * NT, (half + 1) * NT)
                    lx = rtmp.tile([1, NT], F32, tag="rt")
                    nc.scalar.activation(lx, rps[half], AF.Ln)
                    t0 = rtmp.tile([1, NT], F32, tag="rt")
                    nc.scalar.activation(t0, lx, AF.Exp, scale=-0.5)
                    nc.vector.tensor_scalar_mul(st["invx"][:, ns], t0,
                                                c_sb[:, 2:3])
            else:
                # t1h = 1/(16||ye||) = exp(-0.5*ln(ssy) - ln(16))
                for half in range(2):
                    ly = rtmp.tile([1, NT], F32, tag="rt")
                    nc.scalar.activation(ly, rps[half], AF.Ln)
                    t1 = rtmp.tile([1, NT], F32, tag="rt")
                    nc.scalar.activation(t1, ly, AF.Exp, scale=-0.5,
                                         bias=c_sb[:, 3:4])
                    t1h = rtmp.tile([1, NT], F16, tag="rth")
                    nc.vector.tensor_copy(t1h, t1)
                    st["t1h_%d" % half] = t1h

        def x_norms(st):
            # deferred: invx = rw1/16384 * 1/||xe|| over the full [1, T] row
            lx = rtmp.tile([1, T], F32, tag="rtw")
            nc.scalar.activation(lx, st["ssx"], AF.Ln)
            t0 = rtmp.tile([1, T], F32, tag="rtw")
            nc.scalar.activation(t0, lx, AF.Exp, scale=-0.5)
            nc.vector.tensor_scalar_mul(st["invx"], t0, c_sb[:, 2:3])

        def y_norms(st):
            # deferred: t1h = 1/(16||ye||) over the full [1, T] row
            ly = rtmp.tile([1, T], F32, tag="rtw")
            nc.scalar.activation(ly, st["ssy"], AF.Ln)
            t1 = rtmp.tile([1, T], F32, tag="rtw")
            nc.scalar.activation(t1, ly, AF.Exp, scale=-0.5,
                                 bias=c_sb[:, 3:4])
            t1h = rtmp.tile([1, T], F16, tag="rthw")
            nc.vector.tensor_copy(t1h, t1)
            st["t1h_0"] = t1h[:, 0:NT]
            st["t1h_1"] = t1h[:, NT:T]

        def y_pe(st):
            # yn = ye8 * (1/(16||ye||)) broadcast; full-row STT accumulates
            # straight into the ybar sum (no per-half partials)
            b = st["b"]
            ivb16 = bc16p.tile([128, T], F16, tag="bc16")
            for ti in range(NTT):
                ns = slice(ti * NT, (ti + 1) * NT)
                ivb = bcp.tile([128, NT], F32, tag="bc")
                nc.tensor.matmul(ivb, onesr16_sb, st["t1h_%d" % ti],
                                 start=True, stop=True)
                # DVE copy: keeps the ACT queue free for PSUM evacuations
                nc.vector.tensor_copy(ivb16[:, ns], ivb)
            ybf = bat.tile([128, KDP, 2, 1], F32, tag="ybf", name=f"yf{b}")
            for m in range(KD):
                tmp = tmpp.tile([128, T], F16, tag="tmp")
                nc.vector.scalar_tensor_tensor(
                    tmp, st["ye8"][:, m, :], 1.0, ivb16,
                    op0=ALU.mult, op1=ALU.mult,
                    accum_out=ybf[:, m // 2, m % 2, :])
            # padded [.., 2, 16] fp8: dual-row ldweights needs the k-pair
            # step 16B-aligned
            ybar8 = bat.tile([128, KDP, 2, 16], F8, tag="ybar",
                             name=f"yb{b}")
            nc.vector.tensor_copy(ybar8[:, :, :, 0:1], ybf)
            st["ybar8"] = ybar8

        def q_scores(st):
            # q = 256*(xe.ybar) ; scores += q * invx  (consts folded)
            b = st["b"]
            mxp = rows.tile([1, NTT], F32, tag="mxp", name=f"mxp{b}")
            for ti in range(NTT):
                ns = slice(ti * NT, (ti + 1) * NT)
                qps = rpp.tile([1, NT], F32, tag="row")
                for kp in range(KDP):
                    nc.tensor.matmul(qps,
                                     st["ybar8"][:, kp, :, 0:1],
                                     st["xe8"][:, 2 * kp:2 * kp + 2, ns],
                                     start=(kp == 0), stop=(kp == KDP - 1),
                                     perf_mode=DR)
                s0 = rtmp.tile([1, NT], F32, tag="rt")
                nc.vector.tensor_mul(s0, qps, st["invx"][:, ns])
                nc.vector.tensor_add(st["scores"][:, ns], st["scores"][:, ns],
                                     s0)
                nc.vector.reduce_max(mxp[:, ti:ti + 1], st["scores"][:, ns],
                                     axis=AX.X)
            mx = rows.tile([1, 1], F32, tag="mx", name=f"mx{b}")
            nc.vector.reduce_max(mx, mxp, axis=AX.X, negate=True)
            st["mx"] = mx

        def q_exp(st):
            # emitted a few m-blocks after q_scores so the exp's deps are
            # long resolved when the in-order ACT queue reaches it; exp
            # writes the fp16 weights row directly (no extra copy)
            b = st["b"]
            ewh = rows.tile([1, T], F16, tag="ewh", name=f"ew{b}")
            nc.scalar.activation(ewh, st["scores"], AF.Exp, bias=st["mx"])
            st["ewh"] = ewh
            # 1/(64*sum) lands at partition b of smcol (per-partition ACT
            # scale on the final [4, D] correction matmul; 1/SW pre-folded)
            sm = rows.tile([1, 1], F32, tag="sm", name=f"sm{b}")
            nc.vector.reduce_sum(sm, ewh, axis=AX.X)
            nc.vector.tensor_scalar_mul(sm, sm, SW)
            smi = rows.tile([1, 1], F32, tag="smi", name=f"smi{b}")
            nc.vector.reciprocal(smi, sm)
            nc.sync.dma_start(smcol[b:b + 1, :], smi)

        def q_rows(st):
            # tail-only: q contribution to a separate row, BEFORE une has
            # produced the un-part of scores (the DVE muls then hide under
            # une's matmuls)
            b = st["b"]
            qrow = rows.tile([1, T], F32, tag="qrow", name=f"qr{b}")
            for ti in range(NTT):
                ns = slice(ti * NT, (ti + 1) * NT)
                qps = rpp.tile([1, NT], F32, tag="row")
                for kp in range(KDP):
                    nc.tensor.matmul(qps,
                                     st["ybar8"][:, kp, :, 0:1],
                                     st["xe8"][:, 2 * kp:2 * kp + 2, ns],
                                     start=(kp == 0), stop=(kp == KDP - 1),
                                     perf_mode=DR)
                nc.vector.tensor_mul(qrow[:, ns], qps, st["invx"][:, ns])
            st["qrow"] = qrow

        def scores_fin(st):
            b = st["b"]
            nc.vector.tensor_add(st["scores"], st["scores"], st["qrow"])
            mx = rows.tile([1, 1], F32, tag="mx", name=f"mx{b}")
            nc.vector.reduce_max(mx, st["scores"], axis=AX.X, negate=True)
            st["mx"] = mx

        def pass2_w(st, tail=False):
            # V[:, :, b] = sum_t w[t]*h[t]: bcast ew/16 (c16r stationary) then
            # fused DVE multiply+accumulate over h8 = 16*h -> real-scale V.
            # Both halves merged into one [128, T] STT per m (free-axis accum
            # covers the full token range directly).
            b = st["b"]
            wbc16 = bc16p.tile([128, T], F16, tag="bc16w", name=f"wb{b}")
            for ti in range(NTT):
                ns = slice(ti * NT, (ti + 1) * NT)
                wbc = bcp.tile([128, NT], F32, tag="bc")
                nc.tensor.matmul(wbc, c16r_sb, st["ewh"][:, ns],
                                 start=True, stop=True)
                nc.vector.tensor_copy(wbc16[:, ns], wbc)
            vacc = bat.tile([128, KI, 1], F32, tag="vacc", name=f"va{b}")
            for m in range(KI):
                tmp = tmpp.tile([128, T], F16, tag="tmp")
                nc.vector.scalar_tensor_tensor(
                    tmp, st["h8"][:, m, :], 1.0, wbc16,
                    op0=ALU.mult, op1=ALU.mult,
                    accum_out=vacc[:, m, :])
                if tail:
                    # per-k V write unblocks correction matmul k
                    nc.vector.tensor_copy(v16[:, m, b:b + 1], vacc[:, m, :])
                    nc.vector.tensor_scalar_mul(v16l[:, m, b:b + 1],
                                                v16[:, m, b:b + 1],
                                                1.0 / 16.0)
            if not tail:
                nc.vector.tensor_copy(v16[:, :, b:b + 1], vacc)
                nc.vector.tensor_scalar_mul(v16l[:, :, b:b + 1],
                                            v16[:, :, b:b + 1], 1.0 / 16.0)

        def final_correction():
            # out[b, :] = (V[:, b] @ (W2hi + W2lo/16)) / (64*sum_b) + b2
            # k-outer: correction matmuls for k stream as soon as the per-k
            # V writes land (two separate banks, one accumulation region each)
            HD = D // 2
            psc_a = mm2.tile([128, T], F32, tag="mm", name="pc0")
            psc_b = mm2.tile([128, T], F32, tag="mm", name="pc1")
            psc = [psc_a[0:4, 0:HD], psc_b[0:4, 0:HD]]
            for kp in range(KIP):
                for j in range(2):
                    k = 2 * kp + j
                    for h in range(2):
                        hs = slice(h * HD, (h + 1) * HD)
                        nc.tensor.matmul(psc[h], v16[:, k, :],
                                         w2h_sb[:, kp, j, hs],
                                         start=(kp == 0 and j == 0),
                                         stop=False)
                        nc.tensor.matmul(psc[h], v16l[:, k, :],
                                         w2l_sb[:, kp, j, hs],
                                         start=False,
                                         stop=(kp == KIP - 1 and j == 1))
            outf = bat.tile([4, D], F32, tag="outf", name="outf")
            for h in range(2):
                hs = slice(h * HD, (h + 1) * HD)
                nc.scalar.activation(outf[:, hs], psc[h], AF.Identity,
                                     scale=smcol)
                nc.vector.tensor_add(outf[:, hs], outf[:, hs], b2r_sb[:, hs])
            nc.sync.dma_start(out[:, :], outf)

        def prefetch_x(b):
            if b >= bpc:
                return {}
            st_x = xp.tile([128, KI, T], F8, tag="xt", name=f"xt{b}")
            nc.scalar.dma_start(st_x, xt[b])
            return st_x

        def mk(f, *a):
            return lambda: f(*a)

        prev = None
        next_x = first_x
        for b in range(bpc):
            st = alloc_batch(b)
            st["x_sb"] = next_x
            holder = {}
            # fc1 hooks: prev batch's softmax chain + the next x prefetch
            # hide under the 12 dense m-blocks
            todo = {}

            def add_hook(m, f):
                todo.setdefault(m, []).append(f)

            if b + 1 < bpc:
                add_hook(0, lambda bb=b + 1: holder.__setitem__(
                    "x", prefetch_x(bb)))
            if prev is not None:
                # chore train packed early so pass2_w's DVE grind completes
                # during fc2 and leaves the DVE free for the next pw phases
                add_hook(0, mk(y_norms, prev))
                add_hook(1, mk(x_norms, prev))
                add_hook(3, mk(y_pe, prev))
                add_hook(7, mk(q_scores, prev))
                add_hook(9, mk(q_exp, prev))
                add_hook(10, mk(pass2_w, prev))
            hooks = {m: (lambda fs=fs: [f() for f in fs])
                     for m, fs in todo.items()}
            fc1_part(st, hooks)
            fc2_part(st)
            if b < bpc - 1:
                une_part(st)
                pw_part(st, "x", defer=True)
                pw_part(st, "y", defer=True)
            else:
                # last batch: y_pe chores hook into pwx's matmul stream, une
                # goes LAST so the q/invx chain hides under its matmuls;
                # only the exp chain, weighted sum and final correction
                # remain exposed
                pw_part(st, "y")
                pw_part(st, "x", hooks={2: mk(y_pe, st)})
                q_rows(st)
                une_part(st)
            next_x = holder.get("x")
            prev = st
        scores_fin(prev)
        q_exp(prev)
        pass2_w(prev, tail=True)
        final_correction()


_CACHE = {}


def _get_nc():
    if "nc" not in _CACHE:
        nc = build_nc(BPC)
        nc.finalize()
        _CACHE["nc"] = nc
    return _CACHE["nc"]


def _q8(a, scale):
    return (np.asarray(a, np.float32) * scale).astype(ml_dtypes.float8_e4m3)


def _pack_dual(w8):
    """[K, M] fp8 -> [128, K/256, 2, M] dual-row ldweights layout."""
    K, M = w8.shape
    kp = K // 256
    return np.ascontiguousarray(w8.reshape(kp, 2, 128, M).transpose(2, 0, 1, 3))


def _pack_w1(w18):
    """[K=IN, M=IN] fp8 -> m-major [12, 128, 6, 2, 128]."""
    a = w18.reshape(KIP, 2, 128, KI, 128)        # kp, q, p, mo, mi
    return np.ascontiguousarray(a.transpose(3, 2, 0, 1, 4))


def _pack_row(v8):
    """[K] fp8 -> [128, K/256, 2, 16] padded dual layout, value at col 0."""
    K = v8.shape[0]
    kp = K // 256
    outv = np.zeros([128, kp, 2, 16], ml_dtypes.float8_e4m3)
    outv[:, :, :, 0] = v8.reshape(kp, 2, 128).transpose(2, 0, 1)
    return outv


def make_in_maps(x, fc1_w, fc1_b, fc2_w, fc2_b, un_emb_w, un_emb_b,
                 un_red_w, un_red_b, pw_x_w, pw_x_b, pw_y_w, pw_y_b, red_w):
    w2s = np.ascontiguousarray(fc2_w.T).astype(np.float32) * SW
    w2hi = w2s.astype(ml_dtypes.float8_e4m3)
    w2lo = ((w2s - w2hi.astype(np.float32)) * 16.0).astype(
        ml_dtypes.float8_e4m3)
    shared = {
        "w1p": _pack_w1(_q8(np.ascontiguousarray(fc1_w.T), SW)),
        "w2hi": _pack_dual(w2hi),
        "w2lo": _pack_dual(w2lo),
        "wet": _pack_dual(_q8(np.ascontiguousarray(un_emb_w.T), SW)),
        "wxt": _pack_dual(_q8(np.ascontiguousarray(pw_x_w.T), SW)),
        "wyt": _pack_dual(_q8(np.ascontiguousarray(pw_y_w.T), SW)),
        "wr8": _pack_row(_q8(un_red_w[0], SW)),
        "ones8": _pack_row(np.ones([D], np.float32).astype(
            ml_dtypes.float8_e4m3)),
        "b1s": np.asarray(fc1_b, np.float32) * SX,
        "b2s": np.asarray(fc2_b, np.float32) * SX,
        "bes": np.asarray(un_emb_b, np.float32) * SX,
        "bxs": np.asarray(pw_x_b, np.float32) * SX,
        "bys": np.asarray(pw_y_b, np.float32) * SX,
        "consts": np.array([un_red_b[0] * red_w[0], red_w[0] / 1024.0,
                            red_w[1] / 16384.0, -np.log(16.0)], np.float32),
        "onesr16": np.ones([1, 128], np.float16),
        "c16r": np.full([1, 128], 1.0 / 16.0, np.float16),
        "b2rep": np.tile(np.asarray(fc2_b, np.float16)[None, :], (4, 1)),
    }
    in_maps = []
    for c in range(NCORES):
        a = _q8(x[c * BPC:(c + 1) * BPC], SX)          # [bpc, T, IN] fp8
        a = a.reshape(BPC, T, KI, 128).transpose(0, 3, 2, 1)
        in_maps.append({"xt": np.ascontiguousarray(a), **shared})
    return in_maps


def kernel(**inputs) -> np.ndarray:
    inputs = {k: np.asarray(v) for k, v in inputs.items()}
    nc = _get_nc()
    in_maps = make_in_maps(**inputs)
    res = run_bass_kernel_spmd(nc, in_maps, core_ids=list(range(NCORES)))
    return np.concatenate([res.results[c]["out"] for c in range(NCORES)],
                          axis=0)


# revision 57
# speedup vs baseline: 1.0139x; 1.0060x over previous
"""Trainium2 Bass kernel for FGAEmbedder (B=32, T=1024, IN=1536, D=768).

Math (identical to the reference up to float reassociation + fp8 noise;
validated vs the jax reference in numpy at rel_err ~1.1e-2 < 2e-2):
    h  = relu(x @ W1^T + b1)           [B,T,IN]   fp8 (x8/w1 fp8, DR)
    u  = h @ W2^T + b2                 [B,T,D]    fp8
    e  = relu(u @ We^T + be)  ; un = e @ Wr^T + br       (score path)
    xe = u @ Wx^T + bx ; ye = u @ Wy^T + by              (score path)
    pw[t] = (xe[t] . ybar) / ||xe[t]||, ybar = sum_s ye[s]/||ye[s]||
    w  = softmax(rw0*un + rw1*pw)
    out = (sum_t w[t] * h[t]) @ W2^T + b2        <- fc2 is linear, so the
          weighted sum is pushed through W2 (two-plane fp8 hi+lo weights).

Perf structure:
  - every big matmul is fp8 DoubleRow, and each stationary weight block is
    loaded ONCE and used for TWO N=512 moving halves (the redundant second
    LDWEIGHTS of an identical stationary hides completely under the matmul,
    ~213ns/MM vs ~300ns when alternating LDW/MM with distinct weights).
  - fc1's ACT evacuation writes h8 (16*h fp8) directly: no fp16 h tensor, no
    DVE quantize pass; the exact-path weighted sum V runs off h8.
  - row reductions (un-reduce, sum-of-squares for the cosine norms) are fp8
    DoubleRow matmuls with 16B-padded dual stationaries.
  - w1 is packed m-major on the host so the fc1 weight stream arrives in
    exactly the order the PE consumes it; x8 is prefetched one batch ahead.

Sharding: data-parallel over batch, 4 batches per core, no collectives.
"""

import numpy as np
import ml_dtypes

import concourse.bass as bass
import concourse.bacc as bacc
import concourse.mybir as mybir
import concourse.tile as tile
from concourse.bass_utils import run_bass_kernel_spmd

# The kernel's ACT functions are Relu/Identity/Copy/Ln/Exp. Ln and Exp only
# coexist in the "natural_log_exp_and_others" table set, but the greedy set
# chooser maps exp->"exp_and_others" and ln->"natural_log", thrashing
# ACT_TABLE_LOADs (1.5us each) all kernel long. Filter Exp/Ln out of every
# other set (names and order — and therefore act_func_set_ids — unchanged)
# so the chooser lands on the combined set: exactly one load at startup.
_orig_gat = bacc.get_activation_tables


def _gat_single_set(arch):
    tables = _orig_gat(arch)
    AFt = mybir.ActivationFunctionType
    out = {}
    for name, fns in tables.items():
        fns = set(fns)
        if name != "natural_log_exp_and_others":
            fns.discard(AFt.Exp)
            fns.discard(AFt.Ln)
        out[name] = fns
    return out


bacc.get_activation_tables = _gat_single_set

B, T, IN, D = 32, 1024, 1536, 768
NCORES = 8
BPC = B // NCORES        # batches per core
NT = 512                 # matmul moving free dim (one PSUM bank)
NTT = T // NT            # moving halves
KI = IN // 128           # 12 feature tiles of the 1536 dim
KIP = KI // 2            # 6 fp8 double-row k-pairs
KD = D // 128            # 6 feature tiles of the 768 dim
KDP = KD // 2            # 3 fp8 double-row k-pairs

SX = 16.0                # fp8 activation scale
SW = 64.0                # fp8 weight scale

F16 = mybir.dt.float16
F8 = mybir.dt.float8e4
F32 = mybir.dt.float32
AF = mybir.ActivationFunctionType
ALU = mybir.AluOpType
AX = mybir.AxisListType
DR = mybir.MatmulPerfMode.DoubleRow


def build_nc(bpc: int = BPC) -> bass.Bass:
    nc = bacc.Bacc()

    # x8 pre-packed partition-major on the host: [b, p, ko, t] so the per-
    # batch DMA is one fully-contiguous 1.5MB transfer (a (ko p) t gather in
    # 1KB chunks only sustains ~140GB/s and starved the first fc1)
    xt = nc.declare_dram_parameter("xt", [bpc, 128, KI, T], F8, isOutput=False)
    # w1 m-major dual pack: [m, 128, kp, 2, 128]
    w1p = nc.declare_dram_parameter("w1p", [KI, 128, KIP, 2, 128], F8,
                                    isOutput=False)
    w2hi = nc.declare_dram_parameter("w2hi", [128, KIP, 2, D], F8,
                                     isOutput=False)
    w2lo = nc.declare_dram_parameter("w2lo", [128, KIP, 2, D], F8,
                                     isOutput=False)
    wet = nc.declare_dram_parameter("wet", [128, KDP, 2, D], F8,
                                    isOutput=False)
    wxt = nc.declare_dram_parameter("wxt", [128, KDP, 2, D], F8,
                                    isOutput=False)
    wyt = nc.declare_dram_parameter("wyt", [128, KDP, 2, D], F8,
                                    isOutput=False)
    wr8d = nc.declare_dram_parameter("wr8", [128, KDP, 2, 16], F8,
                                     isOutput=False)
    ones8d = nc.declare_dram_parameter("ones8", [128, KDP, 2, 16], F8,
                                       isOutput=False)
    b1d = nc.declare_dram_parameter("b1s", [IN], F32, isOutput=False)   # 16*b1
    b2sd = nc.declare_dram_parameter("b2s", [D], F32, isOutput=False)   # 16*b2
    bed = nc.declare_dram_parameter("bes", [D], F32, isOutput=False)    # 16*be
    bxd = nc.declare_dram_parameter("bxs", [D], F32, isOutput=False)    # 16*bx
    byd = nc.declare_dram_parameter("bys", [D], F32, isOutput=False)    # 16*by
    # consts = [un_red_b*red_w0, red_w0/1024, red_w1/16384, 0]
    cst = nc.declare_dram_parameter("consts", [4], F32, isOutput=False)
    onesr16 = nc.declare_dram_parameter("onesr16", [1, 128], F16,
                                        isOutput=False)
    c16r = nc.declare_dram_parameter("c16r", [1, 128], F16, isOutput=False)
    b2rep = nc.declare_dram_parameter("b2rep", [4, D], F16, isOutput=False)
    out = nc.declare_dram_parameter("out", [bpc, D], F32, isOutput=True)

    with tile.TileContext(nc) as tc:
        _body(nc, tc, bpc, xt, w1p, w2hi, w2lo, wet, wxt, wyt, wr8d, ones8d,
              b1d, b2sd, bed, bxd, byd, cst, onesr16, c16r, b2rep, out)
    return nc


def _body(nc, tc, bpc, xt, w1p, w2hi, w2lo, wet, wxt, wyt, wr8d, ones8d,
          b1d, b2sd, bed, bxd, byd, cst, onesr16, c16r, b2rep, out):
    with (
        tc.tile_pool(name="wpool", bufs=1) as wpool,
        tc.tile_pool(name="xp", bufs=2) as xp,
        tc.tile_pool(name="h8p", bufs=2) as h8p,
        tc.tile_pool(name="u8p", bufs=2) as u8p,
        tc.tile_pool(name="xe8p", bufs=2) as xe8p,
        tc.tile_pool(name="ye8p", bufs=2) as ye8p,
        tc.tile_pool(name="e8p", bufs=1) as e8p,
        tc.tile_pool(name="sqp", bufs=1) as sqp,
        tc.tile_pool(name="tmpp", bufs=2) as tmpp,
        tc.tile_pool(name="rows", bufs=1) as rows,
        tc.tile_pool(name="rtmp", bufs=2) as rtmp,
        tc.tile_pool(name="bat", bufs=1) as bat,
        tc.tile_pool(name="bc16p", bufs=1) as bc16p,
        tc.tile_pool(name="mm2", bufs=2, space="PSUM") as mm2,
        tc.tile_pool(name="rpp", bufs=3, space="PSUM") as rpp,
        tc.tile_pool(name="bcp", bufs=1, space="PSUM") as bcp,
    ):
        # ---- persistent weights / constants ----
        # ordering: the m=0 fc1 block + batch-0 x8 go first so the PE can
        # start ~3us in; the rest of w1 streams m-major exactly in consume
        # order; w2lo (only used at the very end) goes last.
        b1_sb = wpool.tile([128, KI], F32)
        nc.sync.dma_start(b1_sb, b1d.rearrange("(o p) -> p o", p=128))
        onesr16_sb = wpool.tile([1, 128], F16)
        nc.sync.dma_start(onesr16_sb, onesr16[:, :])
        w1_sb = wpool.tile([128, KI, KIP, 2, 128], F8)
        nc.sync.dma_start(w1_sb[:, 0], w1p[0])
        # kp-chunked so fc1's first m-block can start on kp0 ~3us earlier;
        # on the ACT hwdge queue so x streams in parallel with w1 (sync q)
        first_x = xp.tile([128, KI, T], F8, tag="xt")
        for kp in range(KIP):
            nc.scalar.dma_start(first_x[:, 2 * kp:2 * kp + 2, :],
                                xt[0][:, 2 * kp:2 * kp + 2, :])
        for m in range(1, KI):
            nc.sync.dma_start(w1_sb[:, m], w1p[m])
        w2h_sb = wpool.tile([128, KIP, 2, D], F8)
        nc.sync.dma_start(w2h_sb, w2hi[:, :, :, :])
        b2s_sb = wpool.tile([128, KD], F32)
        nc.sync.dma_start(b2s_sb, b2sd.rearrange("(o p) -> p o", p=128))
        we_sb = wpool.tile([128, KDP, 2, D], F8)
        nc.sync.dma_start(we_sb, wet[:, :, :, :])
        be_sb = wpool.tile([128, KD], F32)
        nc.sync.dma_start(be_sb, bed.rearrange("(o p) -> p o", p=128))
        wr_sb = wpool.tile([128, KDP, 2, 16], F8)
        nc.sync.dma_start(wr_sb, wr8d[:, :, :, :])
        wx_sb = wpool.tile([128, KDP, 2, D], F8)
        nc.sync.dma_start(wx_sb, wxt[:, :, :, :])
        bx_sb = wpool.tile([128, KD], F32)
        nc.sync.dma_start(bx_sb, bxd.rearrange("(o p) -> p o", p=128))
        wy_sb = wpool.tile([128, KDP, 2, D], F8)
        nc.sync.dma_start(wy_sb, wyt[:, :, :, :])
        by_sb = wpool.tile([128, KD], F32)
        nc.sync.dma_start(by_sb, byd.rearrange("(o p) -> p o", p=128))
        ones_sb = wpool.tile([128, KDP, 2, 16], F8)
        nc.sync.dma_start(ones_sb, ones8d[:, :, :, :])
        c_sb = wpool.tile([1, 4], F32)
        nc.sync.dma_start(c_sb, cst[None, :])
        c16r_sb = wpool.tile([1, 128], F16)
        nc.sync.dma_start(c16r_sb, c16r[:, :])
        b2r_sb = wpool.tile([4, D], F16)
        nc.sync.dma_start(b2r_sb, b2rep[:, :])
        w2l_sb = wpool.tile([128, KIP, 2, D], F8)
        nc.sync.dma_start(w2l_sb, w2lo[:, :, :, :])

        # shared across batches: per-batch softmax 1/sum at partition b,
        # V (weighted h sums, real scale) and V/16 for the two-plane W2 mm
        smcol = bat.tile([4, 1], F32, tag="smcol", name="smcol")
        v16 = bat.tile([128, KI, 4], F16, tag="v16", name="v16")

        def alloc_batch(b):
            st = {"b": b}
            st["h8"] = h8p.tile([128, KI, T], F8, tag="h8", name=f"h8_{b}")
            st["xe8"] = xe8p.tile([128, KD, T], F8, tag="xe8", name=f"xe_{b}")
            st["ye8"] = ye8p.tile([128, KD, T], F8, tag="ye8", name=f"ye_{b}")
            st["invx"] = rows.tile([1, T], F32, tag="invx", name=f"ix_{b}")
            st["scores"] = rows.tile([1, T], F32, tag="scores", name=f"sc_{b}")
            return st

        def mm_pair(ps, w_slice, mv, kp, nkp):
            """One stationary block, two N=512 moving halves."""
            nc.tensor.matmul(ps[0], w_slice, mv[:, 2 * kp:2 * kp + 2, 0:NT],
                             start=(kp == 0), stop=(kp == nkp - 1),
                             perf_mode=DR)
            nc.tensor.matmul(ps[1], w_slice, mv[:, 2 * kp:2 * kp + 2, NT:T],
                             start=(kp == 0), stop=(kp == nkp - 1),
                             perf_mode=DR)

        def mm_pair2(ps, w_slice, mv, kp, nkp):
            """One stationary block, both halves of a [128, T] 2-bank psum."""
            mm_pair((ps[:, 0:NT], ps[:, NT:T]), w_slice, mv, kp, nkp)

        def fc1_part(st, hooks={}):
            x_sb = st["x_sb"]
            for m in range(KI):
                ps = mm2.tile([128, T], F32, tag="mm")
                for kp in range(KIP):
                    mm_pair2(ps, w1_sb[:, m, kp], x_sb, kp, KIP)
                # h8 = 16*relu(z): psum = 1024*z -> relu(psum/64 + 16*b1);
                # single ACT over both banks halves the evacuation cost
                nc.scalar.activation(st["h8"][:, m, :], ps,
                                     AF.Relu, bias=b1_sb[:, m:m + 1],
                                     scale=1.0 / SW)
                if m in hooks:
                    hooks[m]()

        def fc2_part(st, hooks={}):
            b = st["b"]
            u8 = u8p.tile([128, KD, T], F8, tag="u8", name=f"u8{b}")
            for m in range(KD):
                ps = mm2.tile([128, T], F32, tag="mm")
                for kp in range(KIP):
                    mm_pair2(ps, w2h_sb[:, kp, :, m * 128:(m + 1) * 128],
                            st["h8"], kp, KIP)
                # u8 = 16*u: psum = 1024*u -> psum/64 + 16*b2
                nc.scalar.activation(u8[:, m, :], ps, AF.Identity,
                                     bias=b2s_sb[:, m:m + 1], scale=1.0 / SW)
                if m in hooks:
                    hooks[m]()
            st["u8"] = u8

        def row_pair(rps, w_slice, mv, kp, nkp):
            """DR row matmul on both halves into [1, T] psum pair."""
            nc.tensor.matmul(rps[0][:, 0:NT], w_slice,
                             mv[:, 2 * kp:2 * kp + 2, 0:NT],
                             start=(kp == 0), stop=(kp == nkp - 1),
                             perf_mode=DR)
            nc.tensor.matmul(rps[1][:, 0:NT], w_slice,
                             mv[:, 2 * kp:2 * kp + 2, NT:T],
                             start=(kp == 0), stop=(kp == nkp - 1),
                             perf_mode=DR)

        def une_part(st, hooks={}):
            b = st["b"]
            u8 = st["u8"]
            e8 = e8p.tile([128, KD, T], F8, tag="e8", name=f"e8{b}")
            rps0 = rpp.tile([1, NT], F32, tag="row")
            rps1 = rpp.tile([1, NT], F32, tag="row")
            rps = (rps0, rps1)
            for m in range(KD):
                ps = mm2.tile([128, T], F32, tag="mm")
                for kp in range(KDP):
                    mm_pair2(ps, we_sb[:, kp, :, m * 128:(m + 1) * 128],
                             u8, kp, KDP)
                # e8 = 16*relu(e): psum = 1024*epre -> relu(psum/64 + 16*be)
                nc.scalar.activation(e8[:, m, :], ps, AF.Relu,
                                     bias=be_sb[:, m:m + 1], scale=1.0 / SW)
                if m in hooks:
                    hooks[m]()
                # wr rows trail one k-pair behind the une m-loop
                if m >= 3 and m % 2 == 1:
                    row_pair(rps, wr_sb[:, (m - 3) // 2, :, 0:1], e8,
                             (m - 3) // 2, KDP)
            row_pair(rps, wr_sb[:, KDP - 1, :, 0:1], e8, KDP - 1, KDP)
            # scores = rw0*un + rw0*br ; rps = 1024*un
            nc.scalar.activation(st["scores"][:, 0:NT], rps[0], AF.Identity,
                                 bias=c_sb[:, 0:1], scale=c_sb[:, 1:2])
            nc.scalar.activation(st["scores"][:, NT:T], rps[1], AF.Identity,
                                 bias=c_sb[:, 0:1], scale=c_sb[:, 1:2])

        def pw_part(st, which, hooks={}, defer=False):
            """pwx or pwy: embedding matmuls + fp8 squares + ss row sums.

            With defer=True the ||v||^2 rows are only DVE-copied to SBUF and
            the ln/exp norm chain runs later (x_norms/y_norms hooks inside
            the next batch's fc1, where the ACT queue has slack); inline the
            [1,512] ACT ops would delay the next phase's PSUM evacuations.
            """
            b = st["b"]
            u8 = st["u8"]
            if which == "x":
                w_sb, bias_sb, dst = wx_sb, bx_sb, st["xe8"]
            else:
                w_sb, bias_sb, dst = wy_sb, by_sb, st["ye8"]
            sq8 = sqp.tile([128, KD, T], F8, tag="sq", name=f"sq{which}{b}")
            rps0 = rpp.tile([1, NT], F32, tag="row")
            rps1 = rpp.tile([1, NT], F32, tag="row")
            rps = (rps0, rps1)
            for m in range(KD):
                ps = mm2.tile([128, T], F32, tag="mm")
                for kp in range(KDP):
                    mm_pair2(ps, w_sb[:, kp, :, m * 128:(m + 1) * 128],
                             u8, kp, KDP)
                # dst = 16*v: psum = 1024*v -> psum/64 + 16*bias
                nc.scalar.activation(dst[:, m, :], ps, AF.Identity,
                                     bias=bias_sb[:, m:m + 1], scale=1.0 / SW)
                # sq8 = v^2, alternating engines: DVE STT (dst/256)*dst and
                # ACT Square((dst/16)^2) split the ~1.2us/row cost so neither
                # queue becomes the phase bottleneck
                if m % 2 == 0:
                    nc.vector.scalar_tensor_tensor(
                        sq8[:, m, :], dst[:, m, :], 1.0 / 256.0, dst[:, m, :],
                        op0=ALU.mult, op1=ALU.mult)
                else:
                    nc.scalar.activation(sq8[:, m, :], dst[:, m, :],
                                         AF.Square, scale=1.0 / 16.0)
                if m in hooks:
                    hooks[m]()
                if m >= 3 and m % 2 == 1:
                    row_pair(rps, ones_sb[:, (m - 3) // 2, :, 0:1], sq8,
                             (m - 3) // 2, KDP)
            row_pair(rps, ones_sb[:, KDP - 1, :, 0:1], sq8, KDP - 1, KDP)
            # rps = ||v||^2 (real scale). rsqrt = exp(-0.5*ln(.)): Ln and Exp
            # live in ONE activation table set together with Relu/Identity,
            # so the ACT engine never thrashes ACT_TABLE_LOADs (Sqrt doesn't
            # share a set with Exp).
            if defer:
                ss = rtmp.tile([1, T], F32, tag="ss" + which)
                nc.vector.tensor_copy(ss[:, 0:NT], rps[0])
                nc.vector.tensor_copy(ss[:, NT:T], rps[1])
                st["ss" + which] = ss
            elif which == "x":
                # invx = rw1/16384 * 1/||xe||  (sign of rw1 kept in c_sb)
                for half in range(2):
                    ns = slice(half * NT, (half + 1) * NT)
                    lx = rtmp.tile([1, NT], F32, tag="rt")
                    nc.scalar.activation(lx, rps[half], AF.Ln)
                    t0 = rtmp.tile([1, NT], F32, tag="rt")
                    nc.scalar.activation(t0, lx, AF.Exp, scale=-0.5)
                    nc.vector.tensor_scalar_mul(st["invx"][:, ns], t0,
                                                c_sb[:, 2:3])
            else:
                # t1h = 1/(16||ye||) = exp(-0.5*ln(ssy) - ln(16))
                for half in range(2):
                    ly = rtmp.tile([1, NT], F32, tag="rt")
                    nc.scalar.activation(ly, rps[half], AF.Ln)
                    t1 = rtmp.tile([1, NT], F32, tag="rt")
                    nc.scalar.activation(t1, ly, AF.Exp, scale=-0.5,
                                         bias=c_sb[:, 3:4])
                    t1h = rtmp.tile([1, NT], F16, tag="rth")
                    nc.vector.tensor_copy(t1h, t1)
                    st["t1h_%d" % half] = t1h

        def x_norms(st):
            # deferred: invx = rw1/16384 * 1/||xe|| over the full [1, T] row
            lx = rtmp.tile([1, T], F32, tag="rtw")
            nc.scalar.activation(lx, st["ssx"], AF.Ln)
            t0 = rtmp.tile([1, T], F32, tag="rtw")
            nc.scalar.activation(t0, lx, AF.Exp, scale=-0.5)
            nc.vector.tensor_scalar_mul(st["invx"], t0, c_sb[:, 2:3])

        def y_norms(st):
            # deferred: t1h = 1/(16||ye||) over the full [1, T] row
            ly = rtmp.tile([1, T], F32, tag="rtw")
            nc.scalar.activation(ly, st["ssy"], AF.Ln)
            t1 = rtmp.tile([1, T], F32, tag="rtw")
            nc.scalar.activation(t1, ly, AF.Exp, scale=-0.5,
                                 bias=c_sb[:, 3:4])
            t1h = rtmp.tile([1, T], F16, tag="rthw")
            nc.vector.tensor_copy(t1h, t1)
            st["t1h_0"] = t1h[:, 0:NT]
            st["t1h_1"] = t1h[:, NT:T]

        def y_pe(st):
            # yn = ye8 * (1/(16||ye||)) broadcast; full-row STT accumulates
            # straight into the ybar sum (no per-half partials)
            b = st["b"]
            ivb16 = bc16p.tile([128, T], F16, tag="bc16")
            for ti in range(NTT):
                ns = slice(ti * NT, (ti + 1) * NT)
                ivb = bcp.tile([128, NT], F32, tag="bc")
                nc.tensor.matmul(ivb, onesr16_sb, st["t1h_%d" % ti],
                                 start=True, stop=True)
                # DVE copy: keeps the ACT queue free for PSUM evacuations
                nc.vector.tensor_copy(ivb16[:, ns], ivb)
            ybf = bat.tile([128, KDP, 2, 1], F32, tag="ybf", name=f"yf{b}")
            for m in range(KD):
                tmp = tmpp.tile([128, T], F16, tag="tmp")
                nc.vector.scalar_tensor_tensor(
                    tmp, st["ye8"][:, m, :], 1.0, ivb16,
                    op0=ALU.mult, op1=ALU.mult,
                    accum_out=ybf[:, m // 2, m % 2, :])
            # padded [.., 2, 16] fp8: dual-row ldweights needs the k-pair
            # step 16B-aligned
            ybar8 = bat.tile([128, KDP, 2, 16], F8, tag="ybar",
                             name=f"yb{b}")
            nc.vector.tensor_copy(ybar8[:, :, :, 0:1], ybf)
            st["ybar8"] = ybar8

        def q_scores(st):
            # q = 256*(xe.ybar) ; scores += q * invx  (consts folded)
            b = st["b"]
            mxp = rows.tile([1, NTT], F32, tag="mxp", name=f"mxp{b}")
            for ti in range(NTT):
                ns = slice(ti * NT, (ti + 1) * NT)
                qps = rpp.tile([1, NT], F32, tag="row")
                for kp in range(KDP):
                    nc.tensor.matmul(qps,
                                     st["ybar8"][:, kp, :, 0:1],
                                     st["xe8"][:, 2 * kp:2 * kp + 2, ns],
                                     start=(kp == 0), stop=(kp == KDP - 1),
                                     perf_mode=DR)
                s0 = rtmp.tile([1, NT], F32, tag="rt")
                nc.vector.tensor_mul(s0, qps, st["invx"][:, ns])
                nc.vector.tensor_add(st["scores"][:, ns], st["scores"][:, ns],
                                     s0)
                nc.vector.reduce_max(mxp[:, ti:ti + 1], st["scores"][:, ns],
                                     axis=AX.X)
            mx = rows.tile([1, 1], F32, tag="mx", name=f"mx{b}")
            nc.vector.reduce_max(mx, mxp, axis=AX.X, negate=True)
            st["mx"] = mx

        def q_exp(st):
            # emitted a few m-blocks after q_scores so the exp's deps are
            # long resolved when the in-order ACT queue reaches it; exp
            # writes the fp16 weights row directly (no extra copy)
            b = st["b"]
            ewh = rows.tile([1, T], F16, tag="ewh", name=f"ew{b}")
            nc.scalar.activation(ewh, st["scores"], AF.Exp, bias=st["mx"])
            st["ewh"] = ewh
            # 1/(64*sum) lands at partition b of smcol (per-partition ACT
            # scale on the final [4, D] correction matmul; 1/SW pre-folded)
            sm = rows.tile([1, 1], F32, tag="sm", name=f"sm{b}")
            nc.vector.reduce_sum(sm, ewh, axis=AX.X)
            nc.vector.tensor_scalar_mul(sm, sm, SW)
            smi = rows.tile([1, 1], F32, tag="smi", name=f"smi{b}")
            nc.vector.reciprocal(smi, sm)
            nc.sync.dma_start(smcol[b:b + 1, :], smi)

        def q_rows(st):
            # tail-only: q contribution to a separate row, BEFORE une has
            # produced the un-part of scores (the DVE muls then hide under
            # une's matmuls)
            b = st["b"]
            qrow = rows.tile([1, T], F32, tag="qrow", name=f"qr{b}")
            for ti in range(NTT):
                ns = slice(ti * NT, (ti + 1) * NT)
                qps = rpp.tile([1, NT], F32, tag="row")
                for kp in range(KDP):
                    nc.tensor.matmul(qps,
                                     st["ybar8"][:, kp, :, 0:1],
                                     st["xe8"][:, 2 * kp:2 * kp + 2, ns],
                                     start=(kp == 0), stop=(kp == KDP - 1),
                                     perf_mode=DR)
                nc.vector.tensor_mul(qrow[:, ns], qps, st["invx"][:, ns])
            st["qrow"] = qrow

        def scores_fin(st):
            b = st["b"]
            nc.vector.tensor_add(st["scores"], st["scores"], st["qrow"])
            mx = rows.tile([1, 1], F32, tag="mx", name=f"mx{b}")
            nc.vector.reduce_max(mx, st["scores"], axis=AX.X, negate=True)
            st["mx"] = mx

        def pass2_w(st, tail=False):
            # V[:, :, b] = sum_t w[t]*h[t]: bcast ew/16 (c16r stationary) then
            # fused DVE multiply+accumulate over h8 = 16*h -> real-scale V.
            # Both halves merged into one [128, T] STT per m (free-axis accum
            # covers the full token range directly).
            b = st["b"]
            wbc16 = bc16p.tile([128, T], F16, tag="bc16w", name=f"wb{b}")
            for ti in range(NTT):
                ns = slice(ti * NT, (ti + 1) * NT)
                wbc = bcp.tile([128, NT], F32, tag="bc")
                nc.tensor.matmul(wbc, c16r_sb, st["ewh"][:, ns],
                                 start=True, stop=True)
                nc.vector.tensor_copy(wbc16[:, ns], wbc)
            for m in range(KI):
                tmp = tmpp.tile([128, T], F16, tag="tmp")
                # accumulate straight into the fp16 V column: each per-k
                # write immediately unblocks correction matmul k in the tail
                nc.vector.scalar_tensor_tensor(
                    tmp, st["h8"][:, m, :], 1.0, wbc16,
                    op0=ALU.mult, op1=ALU.mult,
                    accum_out=v16[:, m, b:b + 1])

        def final_correction():
            # out[b, :] = (V[:, b] @ (W2hi + W2lo/16)) / (64*sum_b) + b2
            # k-outer: correction matmuls for k stream as soon as the per-k
            # V writes land (two separate banks, one accumulation region each)
            HD = D // 2
            psc_a = mm2.tile([128, T], F32, tag="mm", name="pc0")
            psc_b = mm2.tile([128, T], F32, tag="mm", name="pc1")
            psc = [psc_a[0:4, 0:HD], psc_b[0:4, 0:HD]]
            for kp in range(KIP):
                for j in range(2):
                    k = 2 * kp + j
                    for h in range(2):
                        hs = slice(h * HD, (h + 1) * HD)
                        nc.tensor.matmul(psc[h], v16[:, k, :],
                                         w2h_sb[:, kp, j, hs],
                                         start=(kp == 0 and j == 0),
                                         stop=False)
                        nc.tensor.matmul(psc[h], v16[:, k, :],
                                         w2l_sb[:, kp, j, hs],
                                         start=False,
                                         stop=(kp == KIP - 1 and j == 1))
            outf = bat.tile([4, D], F32, tag="outf", name="outf")
            for h in range(2):
                hs = slice(h * HD, (h + 1) * HD)
                nc.scalar.activation(outf[:, hs], psc[h], AF.Identity,
                                     scale=smcol)
                nc.vector.tensor_add(outf[:, hs], outf[:, hs], b2r_sb[:, hs])
            nc.sync.dma_start(out[:, :], outf)

        def prefetch_x(b):
            if b >= bpc:
                return {}
            st_x = xp.tile([128, KI, T], F8, tag="xt", name=f"xt{b}")
            nc.scalar.dma_start(st_x, xt[b])
            return st_x

        def mk(f, *a):
            return lambda: f(*a)

        prev = None
        next_x = first_x
        for b in range(bpc):
            st = alloc_batch(b)
            st["x_sb"] = next_x
            holder = {}
            # fc1 hooks: prev batch's softmax chain + the next x prefetch
            # hide under the 12 dense m-blocks
            todo = {}

            def add_hook(m, f):
                todo.setdefault(m, []).append(f)

            if b + 1 < bpc:
                add_hook(0, lambda bb=b + 1: holder.__setitem__(
                    "x", prefetch_x(bb)))
            if prev is not None:
                # chore train packed early so pass2_w's DVE grind completes
                # during fc2 and leaves the DVE free for the next pw phases
                add_hook(0, mk(y_norms, prev))
                add_hook(1, mk(x_norms, prev))
                add_hook(3, mk(y_pe, prev))
                add_hook(7, mk(q_scores, prev))
                add_hook(9, mk(q_exp, prev))
                add_hook(10, mk(pass2_w, prev))
            hooks = {m: (lambda fs=fs: [f() for f in fs])
                     for m, fs in todo.items()}
            fc1_part(st, hooks)
            fc2_part(st)
            if b < bpc - 1:
                une_part(st)
                pw_part(st, "x", defer=True)
                pw_part(st, "y", defer=True)
            else:
                # last batch: y_pe chores hook into pwx's matmul stream, une
                # goes LAST so the q/invx chain hides under its matmuls;
                # only the exp chain, weighted sum and final correction
                # remain exposed
                pw_part(st, "y")
                pw_part(st, "x", hooks={2: mk(y_pe, st)})
                q_rows(st)
                une_part(st)
            next_x = holder.get("x")
            prev = st
        scores_fin(prev)
        q_exp(prev)
        pass2_w(prev, tail=True)
        final_correction()


_CACHE = {}


def _get_nc():
    if "nc" not in _CACHE:
        nc = build_nc(BPC)
        nc.finalize()
        _CACHE["nc"] = nc
    return _CACHE["nc"]


def _q8(a, scale):
    return (np.asarray(a, np.float32) * scale).astype(ml_dtypes.float8_e4m3)


def _pack_dual(w8):
    """[K, M] fp8 -> [128, K/256, 2, M] dual-row ldweights layout."""
    K, M = w8.shape
    kp = K // 256
    return np.ascontiguousarray(w8.reshape(kp, 2, 128, M).transpose(2, 0, 1, 3))


def _pack_w1(w18):
    """[K=IN, M=IN] fp8 -> m-major [12, 128, 6, 2, 128]."""
    a = w18.reshape(KIP, 2, 128, KI, 128)        # kp, q, p, mo, mi
    return np.ascontiguousarray(a.transpose(3, 2, 0, 1, 4))


def _pack_row(v8):
    """[K] fp8 -> [128, K/256, 2, 16] padded dual layout, value at col 0."""
    K = v8.shape[0]
    kp = K // 256
    outv = np.zeros([128, kp, 2, 16], ml_dtypes.float8_e4m3)
    outv[:, :, :, 0] = v8.reshape(kp, 2, 128).transpose(2, 0, 1)
    return outv


def make_in_maps(x, fc1_w, fc1_b, fc2_w, fc2_b, un_emb_w, un_emb_b,
                 un_red_w, un_red_b, pw_x_w, pw_x_b, pw_y_w, pw_y_b, red_w):
    w2s = np.ascontiguousarray(fc2_w.T).astype(np.float32) * SW
    w2hi = w2s.astype(ml_dtypes.float8_e4m3)
    w2lo = (w2s - w2hi.astype(np.float32)).astype(ml_dtypes.float8_e4m3)
    shared = {
        "w1p": _pack_w1(_q8(np.ascontiguousarray(fc1_w.T), SW)),
        "w2hi": _pack_dual(w2hi),
        "w2lo": _pack_dual(w2lo),
        "wet": _pack_dual(_q8(np.ascontiguousarray(un_emb_w.T), SW)),
        "wxt": _pack_dual(_q8(np.ascontiguousarray(pw_x_w.T), SW)),
        "wyt": _pack_dual(_q8(np.ascontiguousarray(pw_y_w.T), SW)),
        "wr8": _pack_row(_q8(un_red_w[0], SW)),
        "ones8": _pack_row(np.ones([D], np.float32).astype(
            ml_dtypes.float8_e4m3)),
        "b1s": np.asarray(fc1_b, np.float32) * SX,
        "b2s": np.asarray(fc2_b, np.float32) * SX,
        "bes": np.asarray(un_emb_b, np.float32) * SX,
        "bxs": np.asarray(pw_x_b, np.float32) * SX,
        "bys": np.asarray(pw_y_b, np.float32) * SX,
        "consts": np.array([un_red_b[0] * red_w[0], red_w[0] / 1024.0,
                            red_w[1] / 16384.0, -np.log(16.0)], np.float32),
        "onesr16": np.ones([1, 128], np.float16),
        "c16r": np.full([1, 128], 1.0 / 16.0, np.float16),
        "b2rep": np.tile(np.asarray(fc2_b, np.float16)[None, :], (4, 1)),
    }
    in_maps = []
    for c in range(NCORES):
        a = _q8(x[c * BPC:(c + 1) * BPC], SX)          # [bpc, T, IN] fp8
        a = a.reshape(BPC, T, KI, 128).transpose(0, 3, 2, 1)
        in_maps.append({"xt": np.ascontiguousarray(a), **shared})
    return in_maps


def kernel(**inputs) -> np.ndarray:
    inputs = {k: np.asarray(v) for k, v in inputs.items()}
    nc = _get_nc()
    in_maps = make_in_maps(**inputs)
    res = run_bass_kernel_spmd(nc, in_maps, core_ids=list(range(NCORES)))
    return np.concatenate([res.results[c]["out"] for c in range(NCORES)],
                          axis=0)


# revision 61
# speedup vs baseline: 1.0164x; 1.0026x over previous
"""Trainium2 Bass kernel for FGAEmbedder (B=32, T=1024, IN=1536, D=768).

Math (identical to the reference up to float reassociation + fp8 noise;
validated vs the jax reference in numpy at rel_err ~1.1e-2 < 2e-2):
    h  = relu(x @ W1^T + b1)           [B,T,IN]   fp8 (x8/w1 fp8, DR)
    u  = h @ W2^T + b2                 [B,T,D]    fp8
    e  = relu(u @ We^T + be)  ; un = e @ Wr^T + br       (score path)
    xe = u @ Wx^T + bx ; ye = u @ Wy^T + by              (score path)
    pw[t] = (xe[t] . ybar) / ||xe[t]||, ybar = sum_s ye[s]/||ye[s]||
    w  = softmax(rw0*un + rw1*pw)
    out = (sum_t w[t] * h[t]) @ W2^T + b2        <- fc2 is linear, so the
          weighted sum is pushed through W2 (two-plane fp8 hi+lo weights).

Perf structure:
  - every big matmul is fp8 DoubleRow, and each stationary weight block is
    loaded ONCE and used for TWO N=512 moving halves (the redundant second
    LDWEIGHTS of an identical stationary hides completely under the matmul,
    ~213ns/MM vs ~300ns when alternating LDW/MM with distinct weights).
  - fc1's ACT evacuation writes h8 (16*h fp8) directly: no fp16 h tensor, no
    DVE quantize pass; the exact-path weighted sum V runs off h8.
  - row reductions (un-reduce, sum-of-squares for the cosine norms) are fp8
    DoubleRow matmuls with 16B-padded dual stationaries.
  - w1 is packed m-major on the host so the fc1 weight stream arrives in
    exactly the order the PE consumes it; x8 is prefetched one batch ahead.

Sharding: data-parallel over batch, 4 batches per core, no collectives.
"""

import numpy as np
import ml_dtypes

import concourse.bass as bass
import concourse.bacc as bacc
import concourse.mybir as mybir
import concourse.tile as tile
from concourse.bass_utils import run_bass_kernel_spmd

# The kernel's ACT functions are Relu/Identity/Copy/Ln/Exp. Ln and Exp only
# coexist in the "natural_log_exp_and_others" table set, but the greedy set
# chooser maps exp->"exp_and_others" and ln->"natural_log", thrashing
# ACT_TABLE_LOADs (1.5us each) all kernel long. Filter Exp/Ln out of every
# other set (names and order — and therefore act_func_set_ids — unchanged)
# so the chooser lands on the combined set: exactly one load at startup.
_orig_gat = bacc.get_activation_tables


def _gat_single_set(arch):
    tables = _orig_gat(arch)
    AFt = mybir.ActivationFunctionType
    out = {}
    for name, fns in tables.items():
        fns = set(fns)
        if name != "natural_log_exp_and_others":
            fns.discard(AFt.Exp)
            fns.discard(AFt.Ln)
        out[name] = fns
    return out


bacc.get_activation_tables = _gat_single_set

B, T, IN, D = 32, 1024, 1536, 768
NCORES = 8
BPC = B // NCORES        # batches per core
NT = 512                 # matmul moving free dim (one PSUM bank)
NTT = T // NT            # moving halves
KI = IN // 128           # 12 feature tiles of the 1536 dim
KIP = KI // 2            # 6 fp8 double-row k-pairs
KD = D // 128            # 6 feature tiles of the 768 dim
KDP = KD // 2            # 3 fp8 double-row k-pairs

SX = 16.0                # fp8 activation scale
SW = 64.0                # fp8 weight scale

F16 = mybir.dt.float16
F8 = mybir.dt.float8e4
F32 = mybir.dt.float32
AF = mybir.ActivationFunctionType
ALU = mybir.AluOpType
AX = mybir.AxisListType
DR = mybir.MatmulPerfMode.DoubleRow


def build_nc(bpc: int = BPC) -> bass.Bass:
    nc = bacc.Bacc()

    # x8 pre-packed partition-major on the host: [b, p, ko, t] so the per-
    # batch DMA is one fully-contiguous 1.5MB transfer (a (ko p) t gather in
    # 1KB chunks only sustains ~140GB/s and starved the first fc1)
    xt = nc.declare_dram_parameter("xt", [bpc, 128, KI, T], F8, isOutput=False)
    # w1 m-major dual pack: [m, 128, kp, 2, 128]
    w1p = nc.declare_dram_parameter("w1p", [KI, 128, KIP, 2, 128], F8,
                                    isOutput=False)
    w2hi = nc.declare_dram_parameter("w2hi", [128, KIP, 2, D], F8,
                                     isOutput=False)
    w2lo = nc.declare_dram_parameter("w2lo", [128, KIP, 2, D], F8,
                                     isOutput=False)
    wet = nc.declare_dram_parameter("wet", [128, KDP, 2, D], F8,
                                    isOutput=False)
    wxt = nc.declare_dram_parameter("wxt", [128, KDP, 2, D], F8,
                                    isOutput=False)
    wyt = nc.declare_dram_parameter("wyt", [128, KDP, 2, D], F8,
                                    isOutput=False)
    wr8d = nc.declare_dram_parameter("wr8", [128, KDP, 2, 16], F8,
                                     isOutput=False)
    ones8d = nc.declare_dram_parameter("ones8", [128, KDP, 2, 16], F8,
                                       isOutput=False)
    b1d = nc.declare_dram_parameter("b1s", [IN], F32, isOutput=False)   # 16*b1
    b2sd = nc.declare_dram_parameter("b2s", [D], F32, isOutput=False)   # 16*b2
    bed = nc.declare_dram_parameter("bes", [D], F32, isOutput=False)    # 16*be
    bxd = nc.declare_dram_parameter("bxs", [D], F32, isOutput=False)    # 16*bx
    byd = nc.declare_dram_parameter("bys", [D], F32, isOutput=False)    # 16*by
    # consts = [un_red_b*red_w0, red_w0/1024, red_w1/16384, 0]
    cst = nc.declare_dram_parameter("consts", [4], F32, isOutput=False)
    onesr16 = nc.declare_dram_parameter("onesr16", [1, 128], F16,
                                        isOutput=False)
    c16r = nc.declare_dram_parameter("c16r", [1, 128], F16, isOutput=False)
    b2rep = nc.declare_dram_parameter("b2rep", [4, D], F16, isOutput=False)
    out = nc.declare_dram_parameter("out", [bpc, D], F32, isOutput=True)

    with tile.TileContext(nc) as tc:
        _body(nc, tc, bpc, xt, w1p, w2hi, w2lo, wet, wxt, wyt, wr8d, ones8d,
              b1d, b2sd, bed, bxd, byd, cst, onesr16, c16r, b2rep, out)
    return nc


def _body(nc, tc, bpc, xt, w1p, w2hi, w2lo, wet, wxt, wyt, wr8d, ones8d,
          b1d, b2sd, bed, bxd, byd, cst, onesr16, c16r, b2rep, out):
    with (
        tc.tile_pool(name="wpool", bufs=1) as wpool,
        tc.tile_pool(name="xp", bufs=2) as xp,
        tc.tile_pool(name="h8p", bufs=2) as h8p,
        tc.tile_pool(name="u8p", bufs=2) as u8p,
        tc.tile_pool(name="xe8p", bufs=2) as xe8p,
        tc.tile_pool(name="ye8p", bufs=2) as ye8p,
        tc.tile_pool(name="e8p", bufs=1) as e8p,
        tc.tile_pool(name="sqp", bufs=1) as sqp,
        tc.tile_pool(name="tmpp", bufs=2) as tmpp,
        tc.tile_pool(name="rows", bufs=1) as rows,
        tc.tile_pool(name="rtmp", bufs=2) as rtmp,
        tc.tile_pool(name="bat", bufs=1) as bat,
        tc.tile_pool(name="bc16p", bufs=1) as bc16p,
        tc.tile_pool(name="mm2", bufs=2, space="PSUM") as mm2,
        tc.tile_pool(name="rpp", bufs=3, space="PSUM") as rpp,
        tc.tile_pool(name="bcp", bufs=1, space="PSUM") as bcp,
    ):
        # ---- persistent weights / constants ----
        # ordering: the m=0 fc1 block + batch-0 x8 go first so the PE can
        # start ~3us in; the rest of w1 streams m-major exactly in consume
        # order; w2lo (only used at the very end) goes last.
        b1_sb = wpool.tile([128, KI], F32)
        nc.sync.dma_start(b1_sb, b1d.rearrange("(o p) -> p o", p=128))
        onesr16_sb = wpool.tile([1, 128], F16)
        nc.sync.dma_start(onesr16_sb, onesr16[:, :])
        w1_sb = wpool.tile([128, KI, KIP, 2, 128], F8)
        nc.sync.dma_start(w1_sb[:, 0], w1p[0])
        # kp-chunked so fc1's first m-block can start on kp0 ~3us earlier;
        # on the ACT hwdge queue so x streams in parallel with w1 (sync q)
        first_x = xp.tile([128, KI, T], F8, tag="xt")
        for kp in range(KIP):
            nc.scalar.dma_start(first_x[:, 2 * kp:2 * kp + 2, :],
                                xt[0][:, 2 * kp:2 * kp + 2, :])
        for m in range(1, KI):
            nc.sync.dma_start(w1_sb[:, m], w1p[m])
        w2h_sb = wpool.tile([128, KIP, 2, D], F8)
        nc.sync.dma_start(w2h_sb, w2hi[:, :, :, :])
        b2s_sb = wpool.tile([128, KD], F32)
        nc.sync.dma_start(b2s_sb, b2sd.rearrange("(o p) -> p o", p=128))
        we_sb = wpool.tile([128, KDP, 2, D], F8)
        nc.sync.dma_start(we_sb, wet[:, :, :, :])
        be_sb = wpool.tile([128, KD], F32)
        nc.sync.dma_start(be_sb, bed.rearrange("(o p) -> p o", p=128))
        wr_sb = wpool.tile([128, KDP, 2, 16], F8)
        nc.sync.dma_start(wr_sb, wr8d[:, :, :, :])
        wx_sb = wpool.tile([128, KDP, 2, D], F8)
        nc.sync.dma_start(wx_sb, wxt[:, :, :, :])
        bx_sb = wpool.tile([128, KD], F32)
        nc.sync.dma_start(bx_sb, bxd.rearrange("(o p) -> p o", p=128))
        wy_sb = wpool.tile([128, KDP, 2, D], F8)
        nc.sync.dma_start(wy_sb, wyt[:, :, :, :])
        by_sb = wpool.tile([128, KD], F32)
        nc.sync.dma_start(by_sb, byd.rearrange("(o p) -> p o", p=128))
        ones_sb = wpool.tile([128, KDP, 2, 16], F8)
        nc.sync.dma_start(ones_sb, ones8d[:, :, :, :])
        c_sb = wpool.tile([1, 4], F32)
        nc.sync.dma_start(c_sb, cst[None, :])
        c16r_sb = wpool.tile([1, 128], F16)
        nc.sync.dma_start(c16r_sb, c16r[:, :])
        b2r_sb = wpool.tile([4, D], F16)
        nc.sync.dma_start(b2r_sb, b2rep[:, :])
        w2l_sb = wpool.tile([128, KIP, 2, D], F8)
        nc.sync.dma_start(w2l_sb, w2lo[:, :, :, :])

        # shared across batches: per-batch softmax 1/sum at partition b,
        # V (weighted h sums, real scale) and V/16 for the two-plane W2 mm
        smcol = bat.tile([4, 1], F32, tag="smcol", name="smcol")
        v16 = bat.tile([128, KI, 4], F16, tag="v16", name="v16")

        def alloc_batch(b):
            st = {"b": b}
            st["h8"] = h8p.tile([128, KI, T], F8, tag="h8", name=f"h8_{b}")
            st["xe8"] = xe8p.tile([128, KD, T], F8, tag="xe8", name=f"xe_{b}")
            st["ye8"] = ye8p.tile([128, KD, T], F8, tag="ye8", name=f"ye_{b}")
            st["invx"] = rows.tile([1, T], F32, tag="invx", name=f"ix_{b}")
            st["scores"] = rows.tile([1, T], F32, tag="scores", name=f"sc_{b}")
            return st

        def mm_pair(ps, w_slice, mv, kp, nkp):
            """One stationary block, two N=512 moving halves."""
            nc.tensor.matmul(ps[0], w_slice, mv[:, 2 * kp:2 * kp + 2, 0:NT],
                             start=(kp == 0), stop=(kp == nkp - 1),
                             perf_mode=DR)
            nc.tensor.matmul(ps[1], w_slice, mv[:, 2 * kp:2 * kp + 2, NT:T],
                             start=(kp == 0), stop=(kp == nkp - 1),
                             perf_mode=DR)

        def mm_pair2(ps, w_slice, mv, kp, nkp):
            """One stationary block, both halves of a [128, T] 2-bank psum."""
            mm_pair((ps[:, 0:NT], ps[:, NT:T]), w_slice, mv, kp, nkp)

        def fc1_part(st, hooks={}):
            x_sb = st["x_sb"]
            for m in range(KI):
                ps = mm2.tile([128, T], F32, tag="mm")
                for kp in range(KIP):
                    mm_pair2(ps, w1_sb[:, m, kp], x_sb, kp, KIP)
                # h8 = 16*relu(z): psum = 1024*z -> relu(psum/64 + 16*b1);
                # single ACT over both banks halves the evacuation cost
                nc.scalar.activation(st["h8"][:, m, :], ps,
                                     AF.Relu, bias=b1_sb[:, m:m + 1],
                                     scale=1.0 / SW)
                if m in hooks:
                    hooks[m]()

        def fc2_part(st, hooks={}):
            b = st["b"]
            u8 = u8p.tile([128, KD, T], F8, tag="u8", name=f"u8{b}")
            for m in range(KD):
                ps = mm2.tile([128, T], F32, tag="mm")
                for kp in range(KIP):
                    mm_pair2(ps, w2h_sb[:, kp, :, m * 128:(m + 1) * 128],
                            st["h8"], kp, KIP)
                # u8 = 16*u: psum = 1024*u -> psum/64 + 16*b2
                nc.scalar.activation(u8[:, m, :], ps, AF.Identity,
                                     bias=b2s_sb[:, m:m + 1], scale=1.0 / SW)
                if m in hooks:
                    hooks[m]()
            st["u8"] = u8

        def row_pair(rps, w_slice, mv, kp, nkp):
            """DR row matmul on both halves into [1, T] psum pair."""
            nc.tensor.matmul(rps[0][:, 0:NT], w_slice,
                             mv[:, 2 * kp:2 * kp + 2, 0:NT],
                             start=(kp == 0), stop=(kp == nkp - 1),
                             perf_mode=DR)
            nc.tensor.matmul(rps[1][:, 0:NT], w_slice,
                             mv[:, 2 * kp:2 * kp + 2, NT:T],
                             start=(kp == 0), stop=(kp == nkp - 1),
                             perf_mode=DR)

        def une_part(st, hooks={}):
            b = st["b"]
            u8 = st["u8"]
            e8 = e8p.tile([128, KD, T], F8, tag="e8", name=f"e8{b}")
            rps0 = rpp.tile([1, NT], F32, tag="row")
            rps1 = rpp.tile([1, NT], F32, tag="row")
            rps = (rps0, rps1)
            for m in range(KD):
                ps = mm2.tile([128, T], F32, tag="mm")
                for kp in range(KDP):
                    mm_pair2(ps, we_sb[:, kp, :, m * 128:(m + 1) * 128],
                             u8, kp, KDP)
                # e8 = 16*relu(e): psum = 1024*epre -> relu(psum/64 + 16*be)
                nc.scalar.activation(e8[:, m, :], ps, AF.Relu,
                                     bias=be_sb[:, m:m + 1], scale=1.0 / SW)
                if m in hooks:
                    hooks[m]()
                # wr rows trail one k-pair behind the une m-loop
                if m >= 3 and m % 2 == 1:
                    row_pair(rps, wr_sb[:, (m - 3) // 2, :, 0:1], e8,
                             (m - 3) // 2, KDP)
            row_pair(rps, wr_sb[:, KDP - 1, :, 0:1], e8, KDP - 1, KDP)
            # scores = rw0*un + rw0*br ; rps = 1024*un
            nc.scalar.activation(st["scores"][:, 0:NT], rps[0], AF.Identity,
                                 bias=c_sb[:, 0:1], scale=c_sb[:, 1:2])
            nc.scalar.activation(st["scores"][:, NT:T], rps[1], AF.Identity,
                                 bias=c_sb[:, 0:1], scale=c_sb[:, 1:2])

        def pw_part(st, which, hooks={}, defer=False):
            """pwx or pwy: embedding matmuls + fp8 squares + ss row sums.

            With defer=True the ||v||^2 rows are only DVE-copied to SBUF and
            the ln/exp norm chain runs later (x_norms/y_norms hooks inside
            the next batch's fc1, where the ACT queue has slack); inline the
            [1,512] ACT ops would delay the next phase's PSUM evacuations.
            """
            b = st["b"]
            u8 = st["u8"]
            if which == "x":
                w_sb, bias_sb, dst = wx_sb, bx_sb, st["xe8"]
            else:
                w_sb, bias_sb, dst = wy_sb, by_sb, st["ye8"]
            sq8 = sqp.tile([128, KD, T], F8, tag="sq", name=f"sq{which}{b}")
            rps0 = rpp.tile([1, NT], F32, tag="row")
            rps1 = rpp.tile([1, NT], F32, tag="row")
            rps = (rps0, rps1)
            for m in range(KD):
                ps = mm2.tile([128, T], F32, tag="mm")
                for kp in range(KDP):
                    mm_pair2(ps, w_sb[:, kp, :, m * 128:(m + 1) * 128],
                             u8, kp, KDP)
                # dst = 16*v: psum = 1024*v -> psum/64 + 16*bias
                nc.scalar.activation(dst[:, m, :], ps, AF.Identity,
                                     bias=bias_sb[:, m:m + 1], scale=1.0 / SW)
                # sq8 = v^2, alternating engines: DVE STT (dst/256)*dst and
                # ACT Square((dst/16)^2) split the ~1.2us/row cost so neither
                # queue becomes the phase bottleneck
                if m % 2 == 0:
                    nc.vector.scalar_tensor_tensor(
                        sq8[:, m, :], dst[:, m, :], 1.0 / 256.0, dst[:, m, :],
                        op0=ALU.mult, op1=ALU.mult)
                else:
                    nc.scalar.activation(sq8[:, m, :], dst[:, m, :],
                                         AF.Square, scale=1.0 / 16.0)
                if m in hooks:
                    hooks[m]()
                if m >= 3 and m % 2 == 1:
                    row_pair(rps, ones_sb[:, (m - 3) // 2, :, 0:1], sq8,
                             (m - 3) // 2, KDP)
            row_pair(rps, ones_sb[:, KDP - 1, :, 0:1], sq8, KDP - 1, KDP)
            # rps = ||v||^2 (real scale). rsqrt = exp(-0.5*ln(.)): Ln and Exp
            # live in ONE activation table set together with Relu/Identity,
            # so the ACT engine never thrashes ACT_TABLE_LOADs (Sqrt doesn't
            # share a set with Exp).
            if defer:
                ss = rtmp.tile([1, T], F32, tag="ss" + which)
                nc.vector.tensor_copy(ss[:, 0:NT], rps[0])
                nc.vector.tensor_copy(ss[:, NT:T], rps[1])
                st["ss" + which] = ss
            elif which == "x":
                # invx = rw1/16384 * 1/||xe||  (sign of rw1 kept in c_sb)
                for half in range(2):
                    ns = slice(half * NT, (half + 1) * NT)
                    lx = rtmp.tile([1, NT], F32, tag="rt")
                    nc.scalar.activation(lx, rps[half], AF.Ln)
                    t0 = rtmp.tile([1, NT], F32, tag="rt")
                    nc.scalar.activation(t0, lx, AF.Exp, scale=-0.5)
                    nc.vector.tensor_scalar_mul(st["invx"][:, ns], t0,
                                                c_sb[:, 2:3])
            else:
                # t1h = 1/(16||ye||) = exp(-0.5*ln(ssy) - ln(16))
                for half in range(2):
                    ly = rtmp.tile([1, NT], F32, tag="rt")
                    nc.scalar.activation(ly, rps[half], AF.Ln)
                    t1 = rtmp.tile([1, NT], F32, tag="rt")
                    nc.scalar.activation(t1, ly, AF.Exp, scale=-0.5,
                                         bias=c_sb[:, 3:4])
                    t1h = rtmp.tile([1, NT], F16, tag="rth")
                    nc.vector.tensor_copy(t1h, t1)
                    st["t1h_%d" % half] = t1h

        def x_norms(st):
            # deferred: invx = rw1/16384 * 1/||xe|| over the full [1, T] row
            lx = rtmp.tile([1, T], F32, tag="rtw")
            nc.scalar.activation(lx, st["ssx"], AF.Ln)
            t0 = rtmp.tile([1, T], F32, tag="rtw")
            nc.scalar.activation(t0, lx, AF.Exp, scale=-0.5)
            nc.vector.tensor_scalar_mul(st["invx"], t0, c_sb[:, 2:3])

        def y_norms(st):
            # deferred: t1h = 1/(16||ye||) over the full [1, T] row
            ly = rtmp.tile([1, T], F32, tag="rtw")
            nc.scalar.activation(ly, st["ssy"], AF.Ln)
            t1 = rtmp.tile([1, T], F32, tag="rtw")
            nc.scalar.activation(t1, ly, AF.Exp, scale=-0.5,
                                 bias=c_sb[:, 3:4])
            t1h = rtmp.tile([1, T], F16, tag="rthw")
            nc.vector.tensor_copy(t1h, t1)
            st["t1h_0"] = t1h[:, 0:NT]
            st["t1h_1"] = t1h[:, NT:T]

        def y_pe(st):
            # yn = ye8 * (1/(16||ye||)) broadcast; full-row STT accumulates
            # straight into the ybar sum (no per-half partials)
            b = st["b"]
            ivb16 = bc16p.tile([128, T], F16, tag="bc16")
            for ti in range(NTT):
                ns = slice(ti * NT, (ti + 1) * NT)
                ivb = bcp.tile([128, NT], F32, tag="bc")
                nc.tensor.matmul(ivb, onesr16_sb, st["t1h_%d" % ti],
                                 start=True, stop=True)
                # DVE copy: keeps the ACT queue free for PSUM evacuations
                nc.vector.tensor_copy(ivb16[:, ns], ivb)
            ybf = bat.tile([128, KDP, 2, 1], F32, tag="ybf", name=f"yf{b}")
            for m in range(KD):
                tmp = tmpp.tile([128, T], F16, tag="tmp")
                nc.vector.scalar_tensor_tensor(
                    tmp, st["ye8"][:, m, :], 1.0, ivb16,
                    op0=ALU.mult, op1=ALU.mult,
                    accum_out=ybf[:, m // 2, m % 2, :])
            # padded [.., 2, 16] fp8: dual-row ldweights needs the k-pair
            # step 16B-aligned
            ybar8 = bat.tile([128, KDP, 2, 16], F8, tag="ybar",
                             name=f"yb{b}")
            nc.vector.tensor_copy(ybar8[:, :, :, 0:1], ybf)
            st["ybar8"] = ybar8

        def q_scores(st):
            # q = 256*(xe.ybar) ; scores += q * invx  (consts folded)
            b = st["b"]
            mxp = rows.tile([1, NTT], F32, tag="mxp", name=f"mxp{b}")
            for ti in range(NTT):
                ns = slice(ti * NT, (ti + 1) * NT)
                qps = rpp.tile([1, NT], F32, tag="row")
                for kp in range(KDP):
                    nc.tensor.matmul(qps,
                                     st["ybar8"][:, kp, :, 0:1],
                                     st["xe8"][:, 2 * kp:2 * kp + 2, ns],
                                     start=(kp == 0), stop=(kp == KDP - 1),
                                     perf_mode=DR)
                s0 = rtmp.tile([1, NT], F32, tag="rt")
                nc.vector.tensor_mul(s0, qps, st["invx"][:, ns])
                nc.vector.tensor_add(st["scores"][:, ns], st["scores"][:, ns],
                                     s0)
                nc.vector.reduce_max(mxp[:, ti:ti + 1], st["scores"][:, ns],
                                     axis=AX.X)
            mx = rows.tile([1, 1], F32, tag="mx", name=f"mx{b}")
            nc.vector.reduce_max(mx, mxp, axis=AX.X, negate=True)
            st["mx"] = mx

        def q_exp(st):
            # emitted a few m-blocks after q_scores so the exp's deps are
            # long resolved when the in-order ACT queue reaches it; exp
            # writes the fp16 weights row directly (no extra copy)
            b = st["b"]
            ewh = rows.tile([1, T], F16, tag="ewh", name=f"ew{b}")
            nc.scalar.activation(ewh, st["scores"], AF.Exp, bias=st["mx"])
            st["ewh"] = ewh
            # 1/(64*sum) lands at partition b of smcol (per-partition ACT
            # scale on the final [4, D] correction matmul; 1/SW pre-folded)
            sm = rows.tile([1, 1], F32, tag="sm", name=f"sm{b}")
            nc.vector.reduce_sum(sm, ewh, axis=AX.X)
            nc.vector.tensor_scalar_mul(sm, sm, SW)
            smi = rows.tile([1, 1], F32, tag="smi", name=f"smi{b}")
            nc.vector.reciprocal(smi, sm)
            nc.sync.dma_start(smcol[b:b + 1, :], smi)

        def q_rows(st):
            # tail-only: q contribution to a separate row, BEFORE une has
            # produced the un-part of scores (the DVE muls then hide under
            # une's matmuls)
            b = st["b"]
            qrow = rows.tile([1, T], F32, tag="qrow", name=f"qr{b}")
            for ti in range(NTT):
                ns = slice(ti * NT, (ti + 1) * NT)
                qps = rpp.tile([1, NT], F32, tag="row")
                for kp in range(KDP):
                    nc.tensor.matmul(qps,
                                     st["ybar8"][:, kp, :, 0:1],
                                     st["xe8"][:, 2 * kp:2 * kp + 2, ns],
                                     start=(kp == 0), stop=(kp == KDP - 1),
                                     perf_mode=DR)
                nc.vector.tensor_mul(qrow[:, ns], qps, st["invx"][:, ns])
            st["qrow"] = qrow

        def scores_fin(st):
            b = st["b"]
            nc.vector.tensor_add(st["scores"], st["scores"], st["qrow"])
            mx = rows.tile([1, 1], F32, tag="mx", name=f"mx{b}")
            nc.vector.reduce_max(mx, st["scores"], axis=AX.X, negate=True)
            st["mx"] = mx

        def pass2_w(st, tail=False):
            # V[:, :, b] = sum_t w[t]*h[t]: bcast ew/16 (c16r stationary) then
            # fused DVE multiply+accumulate over h8 = 16*h -> real-scale V.
            # Both halves merged into one [128, T] STT per m (free-axis accum
            # covers the full token range directly).
            b = st["b"]
            wbc16 = bc16p.tile([128, T], F16, tag="bc16w", name=f"wb{b}")
            for ti in range(NTT):
                ns = slice(ti * NT, (ti + 1) * NT)
                wbc = bcp.tile([128, NT], F32, tag="bc")
                nc.tensor.matmul(wbc, c16r_sb, st["ewh"][:, ns],
                                 start=True, stop=True)
                nc.vector.tensor_copy(wbc16[:, ns], wbc)
            for m in range(KI):
                tmp = tmpp.tile([128, T], F16, tag="tmp")
                # accumulate straight into the fp16 V column: each per-k
                # write immediately unblocks correction matmul k in the tail
                nc.vector.scalar_tensor_tensor(
                    tmp, st["h8"][:, m, :], 1.0, wbc16,
                    op0=ALU.mult, op1=ALU.mult,
                    accum_out=v16[:, m, b:b + 1])

        def final_correction():
            # out[b, :] = (V[:, b] @ (W2hi + W2lo/16)) / (64*sum_b) + b2
            # k-outer: correction matmuls for k stream as soon as the per-k
            # V writes land (two separate banks, one accumulation region each)
            HD = D // 2
            psc_a = mm2.tile([128, T], F32, tag="mm", name="pc0")
            psc_b = mm2.tile([128, T], F32, tag="mm", name="pc1")
            psc = [psc_a[0:4, 0:HD], psc_b[0:4, 0:HD]]
            for kp in range(KIP):
                for j in range(2):
                    k = 2 * kp + j
                    for h in range(2):
                        hs = slice(h * HD, (h + 1) * HD)
                        nc.tensor.matmul(psc[h], v16[:, k, :],
                                         w2h_sb[:, kp, j, hs],
                                         start=(kp == 0 and j == 0),
                                         stop=False)
                        nc.tensor.matmul(psc[h], v16[:, k, :],
                                         w2l_sb[:, kp, j, hs],
                                         start=False,
                                         stop=(kp == KIP - 1 and j == 1))
            outf = bat.tile([4, D], F32, tag="outf", name="outf")
            for h in range(2):
                hs = slice(h * HD, (h + 1) * HD)
                nc.scalar.activation(outf[:, hs], psc[h], AF.Identity,
                                     scale=smcol)
                nc.vector.tensor_add(outf[:, hs], outf[:, hs], b2r_sb[:, hs])
            nc.sync.dma_start(out[:, :], outf)

        def prefetch_x(b):
            if b >= bpc:
                return {}
            st_x = xp.tile([128, KI, T], F8, tag="xt", name=f"xt{b}")
            nc.scalar.dma_start(st_x, xt[b])
            return st_x

        def mk(f, *a):
            return lambda: f(*a)

        prev = None
        next_x = first_x
        for b in range(bpc):
            st = alloc_batch(b)
            st["x_sb"] = next_x
            holder = {}
            # fc1 hooks: prev batch's softmax chain + the next x prefetch
            # hide under the 12 dense m-blocks
            todo = {}

            def add_hook(m, f):
                todo.setdefault(m, []).append(f)

            if b + 1 < bpc:
                add_hook(0, lambda bb=b + 1: holder.__setitem__(
                    "x", prefetch_x(bb)))
            if prev is not None:
                # chore train packed early so pass2_w's DVE grind completes
                # during fc2 and leaves the DVE free for the next pw phases
                add_hook(0, mk(y_norms, prev))
                add_hook(1, mk(x_norms, prev))
                add_hook(3, mk(y_pe, prev))
                add_hook(7, mk(q_scores, prev))
                add_hook(9, mk(q_exp, prev))
                add_hook(10, mk(pass2_w, prev))
            hooks = {m: (lambda fs=fs: [f() for f in fs])
                     for m, fs in todo.items()}
            fc1_part(st, hooks)
            fc2_part(st)
            if b < bpc - 1:
                une_part(st)
                pw_part(st, "x", defer=True)
                pw_part(st, "y", defer=True)
            else:
                # last batch: y_pe chores hook into pwx's matmul stream, une
                # goes LAST so the q/invx chain hides under its matmuls;
                # only the exp chain, weighted sum and final correction
                # remain exposed
                pw_part(st, "y")
                pw_part(st, "x", hooks={2: mk(y_pe, st)})
                q_rows(st)
                une_part(st)
            next_x = holder.get("x")
            prev = st
        scores_fin(prev)
        q_exp(prev)
        pass2_w(prev, tail=True)
        final_correction()


_CACHE = {}


def _get_nc():
    if "nc" not in _CACHE:
        nc = build_nc(BPC)
        nc.finalize()
        _CACHE["nc"] = nc
    return _CACHE["nc"]


def _q8(a, scale):
    return (np.asarray(a, np.float32) * scale).astype(ml_dtypes.float8_e4m3)


def _pack_dual(w8):
    """[K, M] fp8 -> [128, K/256, 2, M] dual-row ldweights layout."""
    K, M = w8.shape
    kp = K // 256
    return np.ascontiguousarray(w8.reshape(kp, 2, 128, M).transpose(2, 0, 1, 3))


def _pack_w1(w18):
    """[K=IN, M=IN] fp8 -> m-major [12, 128, 6, 2, 128]."""
    a = w18.reshape(KIP, 2, 128, KI, 128)        # kp, q, p, mo, mi
    return np.ascontiguousarray(a.transpose(3, 2, 0, 1, 4))


def _pack_row(v8):
    """[K] fp8 -> [128, K/256, 2, 16] padded dual layout, value at col 0."""
    K = v8.shape[0]
    kp = K // 256
    outv = np.zeros([128, kp, 2, 16], ml_dtypes.float8_e4m3)
    outv[:, :, :, 0] = v8.reshape(kp, 2, 128).transpose(2, 0, 1)
    return outv


def make_in_maps(x, fc1_w, fc1_b, fc2_w, fc2_b, un_emb_w, un_emb_b,
                 un_red_w, un_red_b, pw_x_w, pw_x_b, pw_y_w, pw_y_b, red_w):
    w2s = np.ascontiguousarray(fc2_w.T).astype(np.float32) * SW
    w2hi = w2s.astype(ml_dtypes.float8_e4m3)
    w2lo = (w2s - w2hi.astype(np.float32)).astype(ml_dtypes.float8_e4m3)
    shared = {
        "w1p": _pack_w1(_q8(np.ascontiguousarray(fc1_w.T), SW)),
        "w2hi": _pack_dual(w2hi),
        "w2lo": _pack_dual(w2lo),
        "wet": _pack_dual(_q8(np.ascontiguousarray(un_emb_w.T), SW)),
        "wxt": _pack_dual(_q8(np.ascontiguousarray(pw_x_w.T), SW)),
        "wyt": _pack_dual(_q8(np.ascontiguousarray(pw_y_w.T), SW)),
        "wr8": _pack_row(_q8(un_red_w[0], SW)),
        "ones8": _pack_row(np.ones([D], np.float32).astype(
            ml_dtypes.float8_e4m3)),
        "b1s": np.asarray(fc1_b, np.float32) * SX,
        "b2s": np.asarray(fc2_b, np.float32) * SX,
        "bes": np.asarray(un_emb_b, np.float32) * SX,
        "bxs": np.asarray(pw_x_b, np.float32) * SX,
        "bys": np.asarray(pw_y_b, np.float32) * SX,
        "consts": np.array([un_red_b[0] * red_w[0], red_w[0] / 1024.0,
                            red_w[1] / 16384.0, -np.log(16.0)], np.float32),
        "onesr16": np.ones([1, 128], np.float16),
        "c16r": np.full([1, 128], 1.0 / 16.0, np.float16),
        "b2rep": np.tile(np.asarray(fc2_b, np.float16)[None, :], (4, 1)),
    }
    in_maps = []
    for c in range(NCORES):
        a = _q8(x[c * BPC:(c + 1) * BPC], SX)          # [bpc, T, IN] fp8
        a = a.reshape(BPC, T, KI, 128).transpose(0, 3, 2, 1)
        in_maps.append({"xt": np.ascontiguousarray(a), **shared})
    return in_maps


def kernel(**inputs) -> np.ndarray:
    inputs = {k: np.asarray(v) for k, v in inputs.items()}
    nc = _get_nc()
    in_maps = make_in_maps(**inputs)
    res = run_bass_kernel_spmd(nc, in_maps, core_ids=list(range(NCORES)))
    return np.concatenate([res.results[c]["out"] for c in range(NCORES)],
                          axis=0)


# revision 62
# speedup vs baseline: 1.0173x; 1.0009x over previous
"""Trainium2 Bass kernel for FGAEmbedder (B=32, T=1024, IN=1536, D=768).

Math (identical to the reference up to float reassociation + fp8 noise;
validated vs the jax reference in numpy at rel_err ~1.1e-2 < 2e-2):
    h  = relu(x @ W1^T + b1)           [B,T,IN]   fp8 (x8/w1 fp8, DR)
    u  = h @ W2^T + b2                 [B,T,D]    fp8
    e  = relu(u @ We^T + be)  ; un = e @ Wr^T + br       (score path)
    xe = u @ Wx^T + bx ; ye = u @ Wy^T + by              (score path)
    pw[t] = (xe[t] . ybar) / ||xe[t]||, ybar = sum_s ye[s]/||ye[s]||
    w  = softmax(rw0*un + rw1*pw)
    out = (sum_t w[t] * h[t]) @ W2^T + b2        <- fc2 is linear, so the
          weighted sum is pushed through W2 (two-plane fp8 hi+lo weights).

Perf structure:
  - every big matmul is fp8 DoubleRow, and each stationary weight block is
    loaded ONCE and used for TWO N=512 moving halves (the redundant second
    LDWEIGHTS of an identical stationary hides completely under the matmul,
    ~213ns/MM vs ~300ns when alternating LDW/MM with distinct weights).
  - fc1's ACT evacuation writes h8 (16*h fp8) directly: no fp16 h tensor, no
    DVE quantize pass; the exact-path weighted sum V runs off h8.
  - row reductions (un-reduce, sum-of-squares for the cosine norms) are fp8
    DoubleRow matmuls with 16B-padded dual stationaries.
  - w1 is packed m-major on the host so the fc1 weight stream arrives in
    exactly the order the PE consumes it; x8 is prefetched one batch ahead.

Sharding: data-parallel over batch, 4 batches per core, no collectives.
"""

import numpy as np
import ml_dtypes

import concourse.bass as bass
import concourse.bacc as bacc
import concourse.mybir as mybir
import concourse.tile as tile
from concourse.bass_utils import run_bass_kernel_spmd

# The kernel's ACT functions are Relu/Identity/Copy/Ln/Exp. Ln and Exp only
# coexist in the "natural_log_exp_and_others" table set, but the greedy set
# chooser maps exp->"exp_and_others" and ln->"natural_log", thrashing
# ACT_TABLE_LOADs (1.5us each) all kernel long. Filter Exp/Ln out of every
# other set (names and order — and therefore act_func_set_ids — unchanged)
# so the chooser lands on the combined set: exactly one load at startup.
_orig_gat = bacc.get_activation_tables


def _gat_single_set(arch):
    tables = _orig_gat(arch)
    AFt = mybir.ActivationFunctionType
    out = {}
    for name, fns in tables.items():
        fns = set(fns)
        if name != "natural_log_exp_and_others":
            fns.discard(AFt.Exp)
            fns.discard(AFt.Ln)
        out[name] = fns
    return out


bacc.get_activation_tables = _gat_single_set

B, T, IN, D = 32, 1024, 1536, 768
NCORES = 8
BPC = B // NCORES        # batches per core
NT = 512                 # matmul moving free dim (one PSUM bank)
NTT = T // NT            # moving halves
KI = IN // 128           # 12 feature tiles of the 1536 dim
KIP = KI // 2            # 6 fp8 double-row k-pairs
KD = D // 128            # 6 feature tiles of the 768 dim
KDP = KD // 2            # 3 fp8 double-row k-pairs

SX = 16.0                # fp8 activation scale
SW = 64.0                # fp8 weight scale

F16 = mybir.dt.float16
F8 = mybir.dt.float8e4
F32 = mybir.dt.float32
AF = mybir.ActivationFunctionType
ALU = mybir.AluOpType
AX = mybir.AxisListType
DR = mybir.MatmulPerfMode.DoubleRow


def build_nc(bpc: int = BPC) -> bass.Bass:
    nc = bacc.Bacc()

    # x8 pre-packed partition-major on the host: [b, p, ko, t] so the per-
    # batch DMA is one fully-contiguous 1.5MB transfer (a (ko p) t gather in
    # 1KB chunks only sustains ~140GB/s and starved the first fc1)
    xt = nc.declare_dram_parameter("xt", [bpc, 128, KI, T], F8, isOutput=False)
    # w1 m-major dual pack: [m, 128, kp, 2, 128]
    w1p = nc.declare_dram_parameter("w1p", [KI, 128, KIP, 2, 128], F8,
                                    isOutput=False)
    w2hi = nc.declare_dram_parameter("w2hi", [128, KIP, 2, D], F8,
                                     isOutput=False)
    w2lo = nc.declare_dram_parameter("w2lo", [128, KIP, 2, D], F8,
                                     isOutput=False)
    wet = nc.declare_dram_parameter("wet", [128, KDP, 2, D], F8,
                                    isOutput=False)
    wxt = nc.declare_dram_parameter("wxt", [128, KDP, 2, D], F8,
                                    isOutput=False)
    wyt = nc.declare_dram_parameter("wyt", [128, KDP, 2, D], F8,
                                    isOutput=False)
    wr8d = nc.declare_dram_parameter("wr8", [128, KDP, 2, 16], F8,
                                     isOutput=False)
    ones8d = nc.declare_dram_parameter("ones8", [128, KDP, 2, 16], F8,
                                       isOutput=False)
    b1d = nc.declare_dram_parameter("b1s", [IN], F32, isOutput=False)   # 16*b1
    b2sd = nc.declare_dram_parameter("b2s", [D], F32, isOutput=False)   # 16*b2
    bed = nc.declare_dram_parameter("bes", [D], F32, isOutput=False)    # 16*be
    bxd = nc.declare_dram_parameter("bxs", [D], F32, isOutput=False)    # 16*bx
    byd = nc.declare_dram_parameter("bys", [D], F32, isOutput=False)    # 16*by
    # consts = [un_red_b*red_w0, red_w0/1024, red_w1/16384, 0]
    cst = nc.declare_dram_parameter("consts", [4], F32, isOutput=False)
    onesr16 = nc.declare_dram_parameter("onesr16", [1, 128], F16,
                                        isOutput=False)
    c16r = nc.declare_dram_parameter("c16r", [1, 128], F16, isOutput=False)
    b2rep = nc.declare_dram_parameter("b2rep", [4, D], F16, isOutput=False)
    out = nc.declare_dram_parameter("out", [bpc, D], F32, isOutput=True)

    with tile.TileContext(nc) as tc:
        _body(nc, tc, bpc, xt, w1p, w2hi, w2lo, wet, wxt, wyt, wr8d, ones8d,
              b1d, b2sd, bed, bxd, byd, cst, onesr16, c16r, b2rep, out)
    return nc


def _body(nc, tc, bpc, xt, w1p, w2hi, w2lo, wet, wxt, wyt, wr8d, ones8d,
          b1d, b2sd, bed, bxd, byd, cst, onesr16, c16r, b2rep, out):
    with (
        tc.tile_pool(name="wpool", bufs=1) as wpool,
        tc.tile_pool(name="xp", bufs=2) as xp,
        tc.tile_pool(name="h8p", bufs=2) as h8p,
        tc.tile_pool(name="u8p", bufs=2) as u8p,
        tc.tile_pool(name="xe8p", bufs=2) as xe8p,
        tc.tile_pool(name="ye8p", bufs=2) as ye8p,
        tc.tile_pool(name="e8p", bufs=1) as e8p,
        tc.tile_pool(name="sqp", bufs=1) as sqp,
        tc.tile_pool(name="tmpp", bufs=2) as tmpp,
        tc.tile_pool(name="rows", bufs=1) as rows,
        tc.tile_pool(name="rtmp", bufs=2) as rtmp,
        tc.tile_pool(name="bat", bufs=1) as bat,
        tc.tile_pool(name="bc16p", bufs=1) as bc16p,
        tc.tile_pool(name="mm2", bufs=2, space="PSUM") as mm2,
        tc.tile_pool(name="rpp", bufs=3, space="PSUM") as rpp,
        tc.tile_pool(name="bcp", bufs=1, space="PSUM") as bcp,
    ):
        # ---- persistent weights / constants ----
        # ordering: the m=0 fc1 block + batch-0 x8 go first so the PE can
        # start ~3us in; the rest of w1 streams m-major exactly in consume
        # order; w2lo (only used at the very end) goes last.
        b1_sb = wpool.tile([128, KI], F32)
        nc.sync.dma_start(b1_sb, b1d.rearrange("(o p) -> p o", p=128))
        onesr16_sb = wpool.tile([1, 128], F16)
        nc.sync.dma_start(onesr16_sb, onesr16[:, :])
        # one tile per m-block: dependency tracking stays per-block, so an
        # early fc1 LDWEIGHTS can never be gated on the WHOLE w1 stream
        w1m = []
        for m in range(KI):
            w1m_t = wpool.tile([128, KIP, 2, 128], F8, name=f"w1m{m}")
            w1m.append(w1m_t)
        nc.sync.dma_start(w1m[0], w1p[0])
        # kp-chunked so fc1's first m-block can start on kp0 ~3us earlier;
        # on the ACT hwdge queue so x streams in parallel with w1 (sync q)
        first_x = xp.tile([128, KI, T], F8, tag="xt")
        for kp in range(KIP):
            nc.scalar.dma_start(first_x[:, 2 * kp:2 * kp + 2, :],
                                xt[0][:, 2 * kp:2 * kp + 2, :])
        for m in range(1, KI):
            nc.sync.dma_start(w1m[m], w1p[m])
        w2h_sb = wpool.tile([128, KIP, 2, D], F8)
        nc.sync.dma_start(w2h_sb, w2hi[:, :, :, :])
        b2s_sb = wpool.tile([128, KD], F32)
        nc.sync.dma_start(b2s_sb, b2sd.rearrange("(o p) -> p o", p=128))
        we_sb = wpool.tile([128, KDP, 2, D], F8)
        nc.sync.dma_start(we_sb, wet[:, :, :, :])
        be_sb = wpool.tile([128, KD], F32)
        nc.sync.dma_start(be_sb, bed.rearrange("(o p) -> p o", p=128))
        wr_sb = wpool.tile([128, KDP, 2, 16], F8)
        nc.sync.dma_start(wr_sb, wr8d[:, :, :, :])
        wx_sb = wpool.tile([128, KDP, 2, D], F8)
        nc.sync.dma_start(wx_sb, wxt[:, :, :, :])
        bx_sb = wpool.tile([128, KD], F32)
        nc.sync.dma_start(bx_sb, bxd.rearrange("(o p) -> p o", p=128))
        wy_sb = wpool.tile([128, KDP, 2, D], F8)
        nc.sync.dma_start(wy_sb, wyt[:, :, :, :])
        by_sb = wpool.tile([128, KD], F32)
        nc.sync.dma_start(by_sb, byd.rearrange("(o p) -> p o", p=128))
        ones_sb = wpool.tile([128, KDP, 2, 16], F8)
        nc.sync.dma_start(ones_sb, ones8d[:, :, :, :])
        c_sb = wpool.tile([1, 4], F32)
        nc.sync.dma_start(c_sb, cst[None, :])
        c16r_sb = wpool.tile([1, 128], F16)
        nc.sync.dma_start(c16r_sb, c16r[:, :])
        b2r_sb = wpool.tile([4, D], F16)
        nc.sync.dma_start(b2r_sb, b2rep[:, :])
        w2l_sb = wpool.tile([128, KIP, 2, D], F8)
        nc.sync.dma_start(w2l_sb, w2lo[:, :, :, :])

        # shared across batches: per-batch softmax 1/sum at partition b,
        # V (weighted h sums, real scale) and V/16 for the two-plane W2 mm
        smcol = bat.tile([4, 1], F32, tag="smcol", name="smcol")
        v16 = bat.tile([128, KI, 4], F16, tag="v16", name="v16")

        def alloc_batch(b):
            st = {"b": b}
            st["h8"] = h8p.tile([128, KI, T], F8, tag="h8", name=f"h8_{b}")
            st["xe8"] = xe8p.tile([128, KD, T], F8, tag="xe8", name=f"xe_{b}")
            st["ye8"] = ye8p.tile([128, KD, T], F8, tag="ye8", name=f"ye_{b}")
            st["invx"] = rows.tile([1, T], F32, tag="invx", name=f"ix_{b}")
            st["scores"] = rows.tile([1, T], F32, tag="scores", name=f"sc_{b}")
            return st

        def mm_pair(ps, w_slice, mv, kp, nkp):
            """One stationary block, two N=512 moving halves."""
            nc.tensor.matmul(ps[0], w_slice, mv[:, 2 * kp:2 * kp + 2, 0:NT],
                             start=(kp == 0), stop=(kp == nkp - 1),
                             perf_mode=DR)
            nc.tensor.matmul(ps[1], w_slice, mv[:, 2 * kp:2 * kp + 2, NT:T],
                             start=(kp == 0), stop=(kp == nkp - 1),
                             perf_mode=DR)

        def mm_pair2(ps, w_slice, mv, kp, nkp):
            """One stationary block, both halves of a [128, T] 2-bank psum."""
            mm_pair((ps[:, 0:NT], ps[:, NT:T]), w_slice, mv, kp, nkp)

        def fc1_part(st, hooks={}):
            x_sb = st["x_sb"]
            for m in range(KI):
                ps = mm2.tile([128, T], F32, tag="mm")
                for kp in range(KIP):
                    mm_pair2(ps, w1m[m][:, kp], x_sb, kp, KIP)
                # h8 = 16*relu(z): psum = 1024*z -> relu(psum/64 + 16*b1);
                # single ACT over both banks halves the evacuation cost
                nc.scalar.activation(st["h8"][:, m, :], ps,
                                     AF.Relu, bias=b1_sb[:, m:m + 1],
                                     scale=1.0 / SW)
                if m in hooks:
                    hooks[m]()

        def fc2_part(st, hooks={}):
            b = st["b"]
            u8 = u8p.tile([128, KD, T], F8, tag="u8", name=f"u8{b}")
            for m in range(KD):
                ps = mm2.tile([128, T], F32, tag="mm")
                for kp in range(KIP):
                    mm_pair2(ps, w2h_sb[:, kp, :, m * 128:(m + 1) * 128],
                            st["h8"], kp, KIP)
                # u8 = 16*u: psum = 1024*u -> psum/64 + 16*b2
                nc.scalar.activation(u8[:, m, :], ps, AF.Identity,
                                     bias=b2s_sb[:, m:m + 1], scale=1.0 / SW)
                if m in hooks:
                    hooks[m]()
            st["u8"] = u8

        def row_pair(rps, w_slice, mv, kp, nkp):
            """DR row matmul on both halves into [1, T] psum pair."""
            nc.tensor.matmul(rps[0][:, 0:NT], w_slice,
                             mv[:, 2 * kp:2 * kp + 2, 0:NT],
                             start=(kp == 0), stop=(kp == nkp - 1),
                             perf_mode=DR)
            nc.tensor.matmul(rps[1][:, 0:NT], w_slice,
                             mv[:, 2 * kp:2 * kp + 2, NT:T],
                             start=(kp == 0), stop=(kp == nkp - 1),
                             perf_mode=DR)

        def une_part(st, hooks={}):
            b = st["b"]
            u8 = st["u8"]
            e8 = e8p.tile([128, KD, T], F8, tag="e8", name=f"e8{b}")
            rps0 = rpp.tile([1, NT], F32, tag="row")
            rps1 = rpp.tile([1, NT], F32, tag="row")
            rps = (rps0, rps1)
            for m in range(KD):
                ps = mm2.tile([128, T], F32, tag="mm")
                for kp in range(KDP):
                    mm_pair2(ps, we_sb[:, kp, :, m * 128:(m + 1) * 128],
                             u8, kp, KDP)
                # e8 = 16*relu(e): psum = 1024*epre -> relu(psum/64 + 16*be)
                nc.scalar.activation(e8[:, m, :], ps, AF.Relu,
                                     bias=be_sb[:, m:m + 1], scale=1.0 / SW)
                if m in hooks:
                    hooks[m]()
                # wr rows trail one k-pair behind the une m-loop
                if m >= 3 and m % 2 == 1:
                    row_pair(rps, wr_sb[:, (m - 3) // 2, :, 0:1], e8,
                             (m - 3) // 2, KDP)
            row_pair(rps, wr_sb[:, KDP - 1, :, 0:1], e8, KDP - 1, KDP)
            # scores = rw0*un + rw0*br ; rps = 1024*un
            nc.scalar.activation(st["scores"][:, 0:NT], rps[0], AF.Identity,
                                 bias=c_sb[:, 0:1], scale=c_sb[:, 1:2])
            nc.scalar.activation(st["scores"][:, NT:T], rps[1], AF.Identity,
                                 bias=c_sb[:, 0:1], scale=c_sb[:, 1:2])

        def pw_part(st, which, hooks={}, defer=False):
            """pwx or pwy: embedding matmuls + fp8 squares + ss row sums.

            With defer=True the ||v||^2 rows are only DVE-copied to SBUF and
            the ln/exp norm chain runs later (x_norms/y_norms hooks inside
            the next batch's fc1, where the ACT queue has slack); inline the
            [1,512] ACT ops would delay the next phase's PSUM evacuations.
            """
            b = st["b"]
            u8 = st["u8"]
            if which == "x":
                w_sb, bias_sb, dst = wx_sb, bx_sb, st["xe8"]
            else:
                w_sb, bias_sb, dst = wy_sb, by_sb, st["ye8"]
            sq8 = sqp.tile([128, KD, T], F8, tag="sq", name=f"sq{which}{b}")
            rps0 = rpp.tile([1, NT], F32, tag="row")
            rps1 = rpp.tile([1, NT], F32, tag="row")
            rps = (rps0, rps1)
            for m in range(KD):
                ps = mm2.tile([128, T], F32, tag="mm")
                for kp in range(KDP):
                    mm_pair2(ps, w_sb[:, kp, :, m * 128:(m + 1) * 128],
                             u8, kp, KDP)
                # dst = 16*v: psum = 1024*v -> psum/64 + 16*bias
                nc.scalar.activation(dst[:, m, :], ps, AF.Identity,
                                     bias=bias_sb[:, m:m + 1], scale=1.0 / SW)
                # sq8 = v^2, alternating engines: DVE STT (dst/256)*dst and
                # ACT Square((dst/16)^2) split the ~1.2us/row cost so neither
                # queue becomes the phase bottleneck
                if m % 2 == 0:
                    nc.vector.scalar_tensor_tensor(
                        sq8[:, m, :], dst[:, m, :], 1.0 / 256.0, dst[:, m, :],
                        op0=ALU.mult, op1=ALU.mult)
                else:
                    nc.scalar.activation(sq8[:, m, :], dst[:, m, :],
                                         AF.Square, scale=1.0 / 16.0)
                if m in hooks:
                    hooks[m]()
                if m >= 3 and m % 2 == 1:
                    row_pair(rps, ones_sb[:, (m - 3) // 2, :, 0:1], sq8,
                             (m - 3) // 2, KDP)
            row_pair(rps, ones_sb[:, KDP - 1, :, 0:1], sq8, KDP - 1, KDP)
            # rps = ||v||^2 (real scale). rsqrt = exp(-0.5*ln(.)): Ln and Exp
            # live in ONE activation table set together with Relu/Identity,
            # so the ACT engine never thrashes ACT_TABLE_LOADs (Sqrt doesn't
            # share a set with Exp).
            if defer:
                ss = rtmp.tile([1, T], F32, tag="ss" + which)
                nc.vector.tensor_copy(ss[:, 0:NT], rps[0])
                nc.vector.tensor_copy(ss[:, NT:T], rps[1])
                st["ss" + which] = ss
            elif which == "x":
                # invx = rw1/16384 * 1/||xe||  (sign of rw1 kept in c_sb)
                for half in range(2):
                    ns = slice(half * NT, (half + 1) * NT)
                    lx = rtmp.tile([1, NT], F32, tag="rt")
                    nc.scalar.activation(lx, rps[half], AF.Ln)
                    t0 = rtmp.tile([1, NT], F32, tag="rt")
                    nc.scalar.activation(t0, lx, AF.Exp, scale=-0.5)
                    nc.vector.tensor_scalar_mul(st["invx"][:, ns], t0,
                                                c_sb[:, 2:3])
            else:
                # t1h = 1/(16||ye||) = exp(-0.5*ln(ssy) - ln(16))
                for half in range(2):
                    ly = rtmp.tile([1, NT], F32, tag="rt")
                    nc.scalar.activation(ly, rps[half], AF.Ln)
                    t1 = rtmp.tile([1, NT], F32, tag="rt")
                    nc.scalar.activation(t1, ly, AF.Exp, scale=-0.5,
                                         bias=c_sb[:, 3:4])
                    t1h = rtmp.tile([1, NT], F16, tag="rth")
                    nc.vector.tensor_copy(t1h, t1)
                    st["t1h_%d" % half] = t1h

        def x_norms(st):
            # deferred: invx = rw1/16384 * 1/||xe|| over the full [1, T] row
            lx = rtmp.tile([1, T], F32, tag="rtw")
            nc.scalar.activation(lx, st["ssx"], AF.Ln)
            t0 = rtmp.tile([1, T], F32, tag="rtw")
            nc.scalar.activation(t0, lx, AF.Exp, scale=-0.5)
            nc.vector.tensor_scalar_mul(st["invx"], t0, c_sb[:, 2:3])

        def y_norms(st):
            # deferred: t1h = 1/(16||ye||) over the full [1, T] row
            ly = rtmp.tile([1, T], F32, tag="rtw")
            nc.scalar.activation(ly, st["ssy"], AF.Ln)
            t1 = rtmp.tile([1, T], F32, tag="rtw")
            nc.scalar.activation(t1, ly, AF.Exp, scale=-0.5,
                                 bias=c_sb[:, 3:4])
            t1h = rtmp.tile([1, T], F16, tag="rthw")
            nc.vector.tensor_copy(t1h, t1)
            st["t1h_0"] = t1h[:, 0:NT]
            st["t1h_1"] = t1h[:, NT:T]

        def y_pe(st):
            # yn = ye8 * (1/(16||ye||)) broadcast; full-row STT accumulates
            # straight into the ybar sum (no per-half partials)
            b = st["b"]
            ivb16 = bc16p.tile([128, T], F16, tag="bc16")
            for ti in range(NTT):
                ns = slice(ti * NT, (ti + 1) * NT)
                ivb = bcp.tile([128, NT], F32, tag="bc")
                nc.tensor.matmul(ivb, onesr16_sb, st["t1h_%d" % ti],
                                 start=True, stop=True)
                # DVE copy: keeps the ACT queue free for PSUM evacuations
                nc.vector.tensor_copy(ivb16[:, ns], ivb)
            ybf = bat.tile([128, KDP, 2, 1], F32, tag="ybf", name=f"yf{b}")
            for m in range(KD):
                tmp = tmpp.tile([128, T], F16, tag="tmp")
                nc.vector.scalar_tensor_tensor(
                    tmp, st["ye8"][:, m, :], 1.0, ivb16,
                    op0=ALU.mult, op1=ALU.mult,
                    accum_out=ybf[:, m // 2, m % 2, :])
            # padded [.., 2, 16] fp8: dual-row ldweights needs the k-pair
            # step 16B-aligned
            ybar8 = bat.tile([128, KDP, 2, 16], F8, tag="ybar",
                             name=f"yb{b}")
            nc.vector.tensor_copy(ybar8[:, :, :, 0:1], ybf)
            st["ybar8"] = ybar8

        def q_scores(st):
            # q = 256*(xe.ybar) ; scores += q * invx  (consts folded)
            b = st["b"]
            mxp = rows.tile([1, NTT], F32, tag="mxp", name=f"mxp{b}")
            for ti in range(NTT):
                ns = slice(ti * NT, (ti + 1) * NT)
                qps = rpp.tile([1, NT], F32, tag="row")
                for kp in range(KDP):
                    nc.tensor.matmul(qps,
                                     st["ybar8"][:, kp, :, 0:1],
                                     st["xe8"][:, 2 * kp:2 * kp + 2, ns],
                                     start=(kp == 0), stop=(kp == KDP - 1),
                                     perf_mode=DR)
                s0 = rtmp.tile([1, NT], F32, tag="rt")
                nc.vector.tensor_mul(s0, qps, st["invx"][:, ns])
                nc.vector.tensor_add(st["scores"][:, ns], st["scores"][:, ns],
                                     s0)
                nc.vector.reduce_max(mxp[:, ti:ti + 1], st["scores"][:, ns],
                                     axis=AX.X)
            mx = rows.tile([1, 1], F32, tag="mx", name=f"mx{b}")
            nc.vector.reduce_max(mx, mxp, axis=AX.X, negate=True)
            st["mx"] = mx

        def q_exp(st):
            # emitted a few m-blocks after q_scores so the exp's deps are
            # long resolved when the in-order ACT queue reaches it; exp
            # writes the fp16 weights row directly (no extra copy)
            b = st["b"]
            ewh = rows.tile([1, T], F16, tag="ewh", name=f"ew{b}")
            nc.scalar.activation(ewh, st["scores"], AF.Exp, bias=st["mx"])
            st["ewh"] = ewh
            # 1/(64*sum) lands at partition b of smcol (per-partition ACT
            # scale on the final [4, D] correction matmul; 1/SW pre-folded)
            sm = rows.tile([1, 1], F32, tag="sm", name=f"sm{b}")
            nc.vector.reduce_sum(sm, ewh, axis=AX.X)
            nc.vector.tensor_scalar_mul(sm, sm, SW)
            smi = rows.tile([1, 1], F32, tag="smi", name=f"smi{b}")
            nc.vector.reciprocal(smi, sm)
            nc.sync.dma_start(smcol[b:b + 1, :], smi)

        def q_rows(st):
            # tail-only: q contribution to a separate row, BEFORE une has
            # produced the un-part of scores (the DVE muls then hide under
            # une's matmuls)
            b = st["b"]
            qrow = rows.tile([1, T], F32, tag="qrow", name=f"qr{b}")
            for ti in range(NTT):
                ns = slice(ti * NT, (ti + 1) * NT)
                qps = rpp.tile([1, NT], F32, tag="row")
                for kp in range(KDP):
                    nc.tensor.matmul(qps,
                                     st["ybar8"][:, kp, :, 0:1],
                                     st["xe8"][:, 2 * kp:2 * kp + 2, ns],
                                     start=(kp == 0), stop=(kp == KDP - 1),
                                     perf_mode=DR)
                nc.vector.tensor_mul(qrow[:, ns], qps, st["invx"][:, ns])
            st["qrow"] = qrow

        def scores_fin(st):
            b = st["b"]
            nc.vector.tensor_add(st["scores"], st["scores"], st["qrow"])
            mx = rows.tile([1, 1], F32, tag="mx", name=f"mx{b}")
            nc.vector.reduce_max(mx, st["scores"], axis=AX.X, negate=True)
            st["mx"] = mx

        def pass2_w(st, tail=False):
            # V[:, :, b] = sum_t w[t]*h[t]: bcast ew/16 (c16r stationary) then
            # fused DVE multiply+accumulate over h8 = 16*h -> real-scale V.
            # Both halves merged into one [128, T] STT per m (free-axis accum
            # covers the full token range directly).
            b = st["b"]
            wbc16 = bc16p.tile([128, T], F16, tag="bc16w", name=f"wb{b}")
            for ti in range(NTT):
                ns = slice(ti * NT, (ti + 1) * NT)
                wbc = bcp.tile([128, NT], F32, tag="bc")
                nc.tensor.matmul(wbc, c16r_sb, st["ewh"][:, ns],
                                 start=True, stop=True)
                nc.vector.tensor_copy(wbc16[:, ns], wbc)
            for m in range(KI):
                tmp = tmpp.tile([128, T], F16, tag="tmp")
                # accumulate straight into the fp16 V column: each per-k
                # write immediately unblocks correction matmul k in the tail
                nc.vector.scalar_tensor_tensor(
                    tmp, st["h8"][:, m, :], 1.0, wbc16,
                    op0=ALU.mult, op1=ALU.mult,
                    accum_out=v16[:, m, b:b + 1])

        def final_correction():
            # out[b, :] = (V[:, b] @ (W2hi + W2lo/16)) / (64*sum_b) + b2
            # k-outer: correction matmuls for k stream as soon as the per-k
            # V writes land (two separate banks, one accumulation region each)
            HD = D // 2
            psc_a = mm2.tile([128, T], F32, tag="mm", name="pc0")
            psc_b = mm2.tile([128, T], F32, tag="mm", name="pc1")
            psc = [psc_a[0:4, 0:HD], psc_b[0:4, 0:HD]]
            for kp in range(KIP):
                for j in range(2):
                    k = 2 * kp + j
                    for h in range(2):
                        hs = slice(h * HD, (h + 1) * HD)
                        nc.tensor.matmul(psc[h], v16[:, k, :],
                                         w2h_sb[:, kp, j, hs],
                                         start=(kp == 0 and j == 0),
                                         stop=False)
                        nc.tensor.matmul(psc[h], v16[:, k, :],
                                         w2l_sb[:, kp, j, hs],
                                         start=False,
                                         stop=(kp == KIP - 1 and j == 1))
            outf = bat.tile([4, D], F32, tag="outf", name="outf")
            for h in range(2):
                hs = slice(h * HD, (h + 1) * HD)
                nc.scalar.activation(outf[:, hs], psc[h], AF.Identity,
                                     scale=smcol)
                nc.vector.tensor_add(outf[:, hs], outf[:, hs], b2r_sb[:, hs])
            nc.sync.dma_start(out[:, :], outf)

        def prefetch_x(b):
            if b >= bpc:
                return {}
            st_x = xp.tile([128, KI, T], F8, tag="xt", name=f"xt{b}")
            nc.scalar.dma_start(st_x, xt[b])
            return st_x

        def mk(f, *a):
            return lambda: f(*a)

        prev = None
        next_x = first_x
        for b in range(bpc):
            st = alloc_batch(b)
            st["x_sb"] = next_x
            holder = {}
            # fc1 hooks: prev batch's softmax chain + the next x prefetch
            # hide under the 12 dense m-blocks
            todo = {}

            def add_hook(m, f):
                todo.setdefault(m, []).append(f)

            if b + 1 < bpc:
                add_hook(0, lambda bb=b + 1: holder.__setitem__(
                    "x", prefetch_x(bb)))
            if prev is not None:
                # chore train packed early so pass2_w's DVE grind completes
                # during fc2 and leaves the DVE free for the next pw phases
                add_hook(0, mk(y_norms, prev))
                add_hook(1, mk(x_norms, prev))
                add_hook(3, mk(y_pe, prev))
                add_hook(7, mk(q_scores, prev))
                add_hook(9, mk(q_exp, prev))
                add_hook(10, mk(pass2_w, prev))
            hooks = {m: (lambda fs=fs: [f() for f in fs])
                     for m, fs in todo.items()}
            fc1_part(st, hooks)
            fc2_part(st)
            if b < bpc - 1:
                une_part(st)
                pw_part(st, "x", defer=True)
                pw_part(st, "y", defer=True)
            else:
                # last batch: y_pe chores hook into pwx's matmul stream, une
                # goes LAST so the q/invx chain hides under its matmuls;
                # only the exp chain, weighted sum and final correction
                # remain exposed
                pw_part(st, "y")
                pw_part(st, "x", hooks={2: mk(y_pe, st)})
                q_rows(st)
                une_part(st)
            next_x = holder.get("x")
            prev = st
        scores_fin(prev)
        q_exp(prev)
        pass2_w(prev, tail=True)
        final_correction()


_CACHE = {}


def _get_nc():
    if "nc" not in _CACHE:
        nc = build_nc(BPC)
        nc.finalize()
        _CACHE["nc"] = nc
    return _CACHE["nc"]


def _q8(a, scale):
    return (np.asarray(a, np.float32) * scale).astype(ml_dtypes.float8_e4m3)


def _pack_dual(w8):
    """[K, M] fp8 -> [128, K/256, 2, M] dual-row ldweights layout."""
    K, M = w8.shape
    kp = K // 256
    return np.ascontiguousarray(w8.reshape(kp, 2, 128, M).transpose(2, 0, 1, 3))


def _pack_w1(w18):
    """[K=IN, M=IN] fp8 -> m-major [12, 128, 6, 2, 128]."""
    a = w18.reshape(KIP, 2, 128, KI, 128)        # kp, q, p, mo, mi
    return np.ascontiguousarray(a.transpose(3, 2, 0, 1, 4))


def _pack_row(v8):
    """[K] fp8 -> [128, K/256, 2, 16] padded dual layout, value at col 0."""
    K = v8.shape[0]
    kp = K // 256
    outv = np.zeros([128, kp, 2, 16], ml_dtypes.float8_e4m3)
    outv[:, :, :, 0] = v8.reshape(kp, 2, 128).transpose(2, 0, 1)
    return outv


def make_in_maps(x, fc1_w, fc1_b, fc2_w, fc2_b, un_emb_w, un_emb_b,
                 un_red_w, un_red_b, pw_x_w, pw_x_b, pw_y_w, pw_y_b, red_w):
    w2s = np.ascontiguousarray(fc2_w.T).astype(np.float32) * SW
    w2hi = w2s.astype(ml_dtypes.float8_e4m3)
    w2lo = (w2s - w2hi.astype(np.float32)).astype(ml_dtypes.float8_e4m3)
    shared = {
        "w1p": _pack_w1(_q8(np.ascontiguousarray(fc1_w.T), SW)),
        "w2hi": _pack_dual(w2hi),
        "w2lo": _pack_dual(w2lo),
        "wet": _pack_dual(_q8(np.ascontiguousarray(un_emb_w.T), SW)),
        "wxt": _pack_dual(_q8(np.ascontiguousarray(pw_x_w.T), SW)),
        "wyt": _pack_dual(_q8(np.ascontiguousarray(pw_y_w.T), SW)),
        "wr8": _pack_row(_q8(un_red_w[0], SW)),
        "ones8": _pack_row(np.ones([D], np.float32).astype(
            ml_dtypes.float8_e4m3)),
        "b1s": np.asarray(fc1_b, np.float32) * SX,
        "b2s": np.asarray(fc2_b, np.float32) * SX,
        "bes": np.asarray(un_emb_b, np.float32) * SX,
        "bxs": np.asarray(pw_x_b, np.float32) * SX,
        "bys": np.asarray(pw_y_b, np.float32) * SX,
        "consts": np.array([un_red_b[0] * red_w[0], red_w[0] / 1024.0,
                            red_w[1] / 16384.0, -np.log(16.0)], np.float32),
        "onesr16": np.ones([1, 128], np.float16),
        "c16r": np.full([1, 128], 1.0 / 16.0, np.float16),
        "b2rep": np.tile(np.asarray(fc2_b, np.float16)[None, :], (4, 1)),
    }
    in_maps = []
    for c in range(NCORES):
        a = _q8(x[c * BPC:(c + 1) * BPC], SX)          # [bpc, T, IN] fp8
        a = a.reshape(BPC, T, KI, 128).transpose(0, 3, 2, 1)
        in_maps.append({"xt": np.ascontiguousarray(a), **shared})
    return in_maps


def kernel(**inputs) -> np.ndarray:
    inputs = {k: np.asarray(v) for k, v in inputs.items()}
    nc = _get_nc()
    in_maps = make_in_maps(**inputs)
    res = run_bass_kernel_spmd(nc, in_maps, core_ids=list(range(NCORES)))
    return np.concatenate([res.results[c]["out"] for c in range(NCORES)],
                          axis=0)
